# revision 1
# baseline (speedup 1.0000x reference)
"""BiGCN (2-layer bidirectional GCN + global add pool) on 8 Trainium2 NeuronCores.

Strategy (hardcoded for the nn_BiGCN_graphcl problem shapes):
  - Nodes are sharded graph-aligned: core c owns graphs [128c, 128c+128) and
    their (contiguous, batch-sorted) node range, padded to a common NPC.
  - Per direction (td / bu), edges are assigned to the core owning their
    target node.  GCNConv is computed as
        out = dinv * (scatter_add(hn[src], dst) + hn) + b,   hn = dinv * (x @ W)
    so no per-edge scaling is needed on device.
  - The hn table ([8*NPC, 128] bf16) is AllGathered between layers; each core
    gathers rows for its edge shard with dma_gather (256B rows), builds a
    staircase one-hot with a DVE is_equal against an iota constant, and
    segment-sums on the TensorEngine into per-window (128-node) PSUM tiles.
  - The SPMD program is identical on all cores: all per-core variation lives
    in uploaded index/data tensors; run lengths are padded to the max across
    cores (pad slots gather row 0 of the block and carry dstloc=-1 so their
    one-hot column is zero).
  - Graph pooling is a second one-hot matmul into a [128 graphs, 128] PSUM
    tile; the host just concatenates the 8 per-core [128, 256] outputs.
"""

import math
import numpy as np
import ml_dtypes

BF16 = ml_dtypes.bfloat16

# ---------------------------------------------------------------- problem cfg
FULL_CFG = dict(
    N=100000, E=1600000, IN_FEATS=256, HIDDEN=128, OUT_FEATS=128,
    NUM_GRAPHS=1024, N_CORES=8, SW=8, NBLK=4,
)


def _round_up(x, m):
    return (x + m - 1) // m * m


# =====================================================================
# Host-side metadata construction
# =====================================================================

def build_partition(batch, cfg, deg_td=None, deg_bu=None):
    """Graph-aligned node partition. Returns dict with per-core node ranges.

    If degree arrays are given, each core's local node order is permuted so
    that per-window (128-node) degree sums cluster just under multiples of
    4*128 edges per (window, src-block) run, minimizing ceil-128 padding."""
    N, C, G = cfg["N"], cfg["N_CORES"], cfg["NUM_GRAPHS"]
    gpc = G // C  # graphs per core
    starts = np.searchsorted(batch, np.arange(0, G + 1, gpc))
    counts = np.diff(starts)
    NPC = max(128, _round_up(int(counts.max()), 128))
    W = NPC // 128
    node_core = np.searchsorted(starts[1:], np.arange(N), side="right")
    node_local = np.arange(N) - starts[node_core]

    if deg_td is not None:
        NBLK = cfg["NBLK"]
        MARGIN = 45 * NBLK  # leave room for cross-core/block-split variance
        for c in range(C):
            lo, hi = starts[c], starts[c + 1]
            cnt = hi - lo
            dt = deg_td[lo:hi].astype(np.int64)
            db = deg_bu[lo:hi].astype(np.int64)
            order = np.argsort(-(dt + db), kind="stable")
            tg_t = np.full(W, dt.sum() / W)
            tg_b = np.full(W, db.sum() / W)
            rem_t = tg_t.astype(np.float64).copy()
            rem_b = tg_b.astype(np.float64).copy()
            room = np.full(W, 128, np.int64)
            assign = np.empty(cnt, np.int64)
            for j in order:
                score = np.minimum(rem_t - dt[j], rem_b - db[j])
                score[room <= 0] = -np.inf
                w = int(np.argmax(score))
                assign[j] = w
                rem_t[w] -= dt[j]
                rem_b[w] -= db[j]
                room[w] -= 1
            # positions: window-major order
            slot_in_w = np.zeros(W, np.int64)
            newloc = np.empty(cnt, np.int64)
            for j in range(cnt):
                w = assign[j]
                newloc[j] = w * 128 + slot_in_w[w]
                slot_in_w[w] += 1
            node_local[lo:hi] = newloc

    # ---- chunk decomposition: 4 window-chunks, sized so per-(window, chunk)
    # gather runs land just under multiples of 128, and each chunk's block of
    # 8*128*w_q table rows stays within int16 index range. ----
    NBLK = cfg["NBLK"]
    mean_w = max(1.0, (deg_td.sum() + deg_bu.sum()) / (2.0 * C * W)) if deg_td is not None else 128.0
    wmax = min(W, (32767 // (128 * C)))

    def padfrac(wb):
        r = wb / W * mean_w  # mean edges per (window, this-chunk) run
        if r <= 0:
            return 0.0
        margin = 1.6 * np.sqrt(r) + 6
        gslots = 128 * np.ceil((r + margin) / 128)
        return (gslots - r) * 1.0

    best = None
    for w1 in range(1, wmax + 1):
        for w2 in range(w1, wmax + 1):
            for w3 in range(w2, wmax + 1):
                w4 = W - w1 - w2 - w3
                if w4 < w3 or w4 > wmax:
                    continue
                cost = padfrac(w1) + padfrac(w2) + padfrac(w3) + padfrac(w4)
                if best is None or cost < best[0]:
                    best = (cost, (w1, w2, w3, w4))
    ws = list(best[1]) if best else [W]
    # early chunks smaller -> earlier AG pipelining
    cw = np.concatenate([[0], np.cumsum(ws)])
    assert cw[-1] == W

    chunk_of_w = np.searchsorted(cw[1:], np.arange(W), side="right")
    q = chunk_of_w[np.minimum(node_local // 128, W - 1)]
    rpr = 128 * np.diff(cw)  # rows per rank per chunk
    base = np.concatenate([[0], np.cumsum(rpr * C)])
    table_row = base[q] + node_core * rpr[q] + (node_local - 128 * cw[q])
    bounds = [int(b) for b in base]
    return dict(starts=starts, counts=counts, NPC=NPC, gpc=gpc,
                node_core=node_core.astype(np.int64),
                node_local=node_local.astype(np.int64),
                table_row=table_row.astype(np.int64),
                cw=cw, bounds=bounds)


def build_direction_meta(gather_nodes, target_nodes, part, cfg):
    """Build per-core gather index / dstloc arrays and the uniform group
    structure for one edge direction.

    gather_nodes[e]: node whose table row is gathered for edge e.
    target_nodes[e]: node receiving the contribution.
    """
    N, C = cfg["N"], cfg["N_CORES"]
    SW, NBLK = cfg["SW"], cfg["NBLK"]
    NPC = part["NPC"]
    W = NPC // 128
    NS = (W + SW - 1) // SW
    R = C * NPC

    deg = np.bincount(target_nodes, minlength=N).astype(np.float64) + 1.0

    bounds = part["bounds"]
    assert len(bounds) == NBLK + 1
    assert all(bounds[i + 1] - bounds[i] <= 32767 for i in range(NBLK))
    bounds_arr = np.array(bounds[1:-1])

    tr_g = part["table_row"][gather_nodes]
    t_core = part["node_core"][target_nodes]
    t_local = part["node_local"][target_nodes]
    lw = t_local // 128          # window
    dloc = t_local % 128         # position within window
    blk = np.searchsorted(bounds_arr, tr_g, side="right")
    idxv = tr_g - np.array(bounds[:-1])[blk]
    sup = lw // SW

    # per (core, s, b, w) counts -> uniform G
    keyW = (sup * NBLK + blk) * W + lw  # key within a core
    nkeys = NS * NBLK * W
    counts = np.zeros((C, nkeys), np.int64)
    for c in range(C):
        m = t_core == c
        counts[c] = np.bincount(keyW[m], minlength=nkeys)
    max_counts = counts.max(axis=0).reshape(NS, NBLK, W)

    G = np.ceil(max_counts / 128).astype(np.int64)  # groups per (s,b,w)
    # ensure every window has at least one group (psum must be written)
    for s in range(NS):
        w_lo, w_hi = s * SW, min((s + 1) * SW, W)
        for w in range(w_lo, w_hi):
            if G[s, :, w].sum() == 0:
                G[s, 0, w] = 1
        G[s, :, :w_lo] = 0
        G[s, :, w_hi:] = 0

    # structure: per (s,b): window col bases, totals
    struct = []
    for s in range(NS):
        w_lo, w_hi = s * SW, min((s + 1) * SW, W)
        for b in range(NBLK):
            g_list = G[s, b, w_lo:w_hi]
            base = np.concatenate([[0], np.cumsum(g_list)])
            struct.append(dict(s=s, b=b, w_lo=w_lo, w_hi=w_hi,
                               g_list=g_list, g_base=base,
                               G=int(g_list.sum())))
    # global column offsets
    offG = 0
    off16 = 0
    for sb in struct:
        sb["offG"] = offG
        sb["off16"] = off16
        offG += sb["G"]
        off16 += sb["G"] * 8  # 128 slots / 16
    CG = offG
    Gmax = max((sb["G"] for sb in struct), default=1)

    # per-edge slot assignment (per core)
    idx_all = np.zeros((C, 128, CG * 8), np.int16)
    dloc_all = np.full((C, 128, CG), -1.0, BF16)
    # precompute slot base for each (s,b,w): global slot start
    slot_base = np.zeros((NS, NBLK, W), np.int64)
    for sb in struct:
        s, b = sb["s"], sb["b"]
        for i, w in enumerate(range(sb["w_lo"], sb["w_hi"])):
            slot_base[s, b, w] = (sb["offG"] + sb["g_base"][i]) * 128

    for c in range(C):
        m = t_core == c
        k = keyW[m]
        order = np.argsort(k, kind="stable")
        ks = k[order]
        # rank within each run
        run_start = np.searchsorted(ks, np.arange(nkeys))
        rank = np.arange(len(ks)) - run_start[ks]
        sb_s = ks // (NBLK * W)
        sb_b = (ks // W) % NBLK
        sb_w = ks % W
        slot = slot_base[sb_s, sb_b, sb_w] + rank
        iv = idxv[m][order]
        dv = dloc[m][order]
        # idx wrapped layout: slot j -> (j%16, j//16), replicated x8
        prow = slot % 16
        pcol = slot // 16
        tmp = np.zeros((16, CG * 8), np.int16)
        tmp[prow, pcol] = iv.astype(np.int16)
        idx_all[c] = np.tile(tmp, (8, 1))
        dloc_all[c, slot % 128, slot // 128] = dv.astype(BF16)

    return dict(deg=deg, struct=struct, CG=CG, Gmax=Gmax, NS=NS, W=W,
                bounds=bounds, idx_all=idx_all, dloc_all=dloc_all)


def build_all_inputs(x, edge_index, batch, Ws, bs, cfg):
    """Produce per-core in_maps plus structural metadata."""
    C = cfg["N_CORES"]
    N = cfg["N"]
    src = np.asarray(edge_index[0])
    dst = np.asarray(edge_index[1])
    part = build_partition(batch, cfg,
                           deg_td=np.bincount(dst, minlength=N),
                           deg_bu=np.bincount(src, minlength=N))
    NPC = part["NPC"]
    W = NPC // 128

    td = build_direction_meta(src, dst, part, cfg)   # gather src row, scatter to dst
    bu = build_direction_meta(dst, src, part, cfg)   # reversed

    Gmax = max(td["Gmax"], bu["Gmax"])
    iota_rep = np.tile(np.arange(128, dtype=np.float32), Gmax)[None, :].repeat(128, 0).astype(BF16)

    # per-core tensors
    in_maps = []
    xT_full = np.ascontiguousarray(np.asarray(x).T)  # [IN, N]
    batch_np = np.asarray(batch)
    for c in range(C):
        lo, hi = part["starts"][c], part["starts"][c + 1]
        cnt = hi - lo
        li = part["node_local"][lo:hi]
        xT = np.zeros((cfg["IN_FEATS"], NPC), BF16)
        xT[:, li] = xT_full[:, lo:hi].astype(BF16)
        deg_t = np.ones((128, W), np.float32)
        deg_b = np.ones((128, W), np.float32)
        deg_t[li % 128, li // 128] = td["deg"][lo:hi].astype(np.float32)
        deg_b[li % 128, li // 128] = bu["deg"][lo:hi].astype(np.float32)
        bl = np.full((128, W), -1.0, BF16)
        bl[li % 128, li // 128] = (batch_np[lo:hi] - c * part["gpc"]).astype(BF16)
        im = dict(
            xT=xT, ident=np.eye(128, dtype=BF16),
            deg_td=deg_t, deg_bu=deg_b, batchloc=bl, iota_rep=iota_rep,
            idx_td=td["idx_all"][c], idx_bu=bu["idx_all"][c],
            dstloc_td=td["dloc_all"][c], dstloc_bu=bu["dloc_all"][c],
            W_td1=Ws[0].astype(BF16), W_bu1=Ws[2].astype(BF16),
            W_td2=Ws[1].astype(BF16), W_bu2=Ws[3].astype(BF16),
            b_td1=np.tile(bs[0][None, :], (128, 1)).astype(np.float32),
            b_td2=np.tile(bs[1][None, :], (128, 1)).astype(np.float32),
            b_bu1=np.tile(bs[2][None, :], (128, 1)).astype(np.float32),
            b_bu2=np.tile(bs[3][None, :], (128, 1)).astype(np.float32),
        )
        in_maps.append(im)
    meta = dict(part=part, td=td, bu=bu, Gmax=Gmax, NPC=NPC, W=W, cfg=cfg)
    return in_maps, meta


# =====================================================================
# Bass program
# =====================================================================

def build_bass(meta):
    import concourse.bacc as bacc
    import concourse.mybir as mybir
    import concourse.tile as tile

    cfg = meta["cfg"]
    C = cfg["N_CORES"]
    NPC, W, Gmax = meta["NPC"], meta["W"], meta["Gmax"]
    IN, HID = cfg["IN_FEATS"], cfg["HIDDEN"]
    NBLK = cfg["NBLK"]
    f32, bf16, i16 = mybir.dt.float32, mybir.dt.bfloat16, mybir.dt.int16

    nc = bacc.Bacc("TRN2", target_bir_lowering=False, debug=False, num_devices=C,
                   num_swdge_queues=4)

    # ---- I/O ----
    ten = {}
    def inp(name, shape, dt):
        ten[name] = nc.dram_tensor(name, shape, dt, kind="ExternalInput")
        return ten[name]

    inp("xT", [IN, NPC], bf16)
    inp("deg_td", [128, W], f32); inp("deg_bu", [128, W], f32)
    inp("batchloc", [128, W], bf16)
    inp("iota_rep", [128, Gmax * 128], bf16)
    inp("ident", [128, 128], bf16)
    for d in ("td", "bu"):
        m = meta[d]
        inp(f"idx_{d}", [128, m["CG"] * 8], i16)
        inp(f"dstloc_{d}", [128, m["CG"]], bf16)
        inp(f"W_{d}1", [IN, HID], bf16)
        inp(f"W_{d}2", [HID, HID], bf16)
        inp(f"b_{d}1", [128, HID], f32)
        inp(f"b_{d}2", [128, HID], f32)
    out_t = nc.dram_tensor("out", [128, 2 * HID], f32, kind="ExternalOutput")
    dbg = meta.get("dbg")
    if dbg:
        dbg_h1 = {d: nc.dram_tensor(f"dbg_h1_{d}", [NPC, HID], f32, kind="ExternalOutput")
                  for d in ("td", "bu")}
        dbg_m = {d: nc.dram_tensor(f"dbg_m_{d}", [NPC, HID], f32, kind="ExternalOutput")
                 for d in ("td", "bu")}

    # internal DRAM: AG inputs + tables
    ag_in, table = {}, {}
    for d in ("td", "bu"):
        for l in (1, 2):
            ag_in[d, l] = nc.dram_tensor(f"agin_{d}{l}", [NPC, HID], bf16, kind="Internal")
            table[d, l] = nc.dram_tensor(f"table_{d}{l}", [C * NPC, HID], bf16,
                                         kind="Internal", addr_space="Shared")

    rg = [list(range(C))]

    from contextlib import ExitStack
    with tile.TileContext(nc) as tc, ExitStack() as stack:
        def pool(name, bufs, space="SBUF"):
            return stack.enter_context(tc.tile_pool(name=name, bufs=bufs, space=space))

        const = pool("const", 1)
        xt_p = pool("xt", 6)
        hn_p = pool("hn", 4)                 # hn tiles to DRAM
        idx_p = pool("idx", 4)
        dl_p = pool("dl", 4)
        gat_p = pool("gat", 5)               # gathered edge tiles
        oh_p = pool("oh", 3)                 # one-hot tiles
        win_p = pool("win", 6, "PSUM")       # window psum, 4 windows/bank
        epi_p = pool("epi", 6)               # epilogue sbuf tiles
        h1_p = pool("h1", 4)
        t_p = pool("tt", 4)                  # transposes
        po_p = pool("po", 4)                 # pool one-hot
        outp = pool("outp", 1)
        hps_cm = tc.tile_pool(name="hps", bufs=2, space="PSUM")
        hps_p = hps_cm.__enter__()

        # ---- constants in SBUF ----
        iota = const.tile([128, Gmax * 128], bf16, tag="iota")
        nc.sync.dma_start(iota[:], ten["iota_rep"][:])
        Wt = {}
        for d in ("td", "bu"):
            for l, k in ((1, IN), (2, HID)):
                chunks = []
                for kk in range(k // 128):
                    t = const.tile([128, HID], bf16, tag=f"W_{d}{l}_{kk}", name=f"W_{d}{l}_{kk}")
                    nc.sync.dma_start(t[:], ten[f"W_{d}{l}"][kk * 128:(kk + 1) * 128, :])
                    chunks.append(t)
                Wt[d, l] = chunks
        bt = {}
        for d in ("td", "bu"):
            for l in (1, 2):
                t = const.tile([128, HID], f32, tag=f"b_{d}{l}", name=f"bt_{d}{l}")
                nc.sync.dma_start(t[:], ten[f"b_{d}{l}"][:])
                bt[d, l] = t
        zrow = const.tile([1, 512], bf16, tag="zrow")
        nc.gpsimd.memset(zrow[:], 0.0)
        ident = const.tile([128, 128], bf16, tag="ident")
        nc.sync.dma_start(ident[:], ten["ident"][:])
        batchloc = const.tile([128, W], bf16, tag="batchloc")
        nc.sync.dma_start(batchloc[:], ten["batchloc"][:])

        dinv = {}
        for d in ("td", "bu"):
            degt = const.tile([128, W], f32, tag=f"deg_{d}", name=f"degt_{d}")
            nc.sync.dma_start(degt[:], ten[f"deg_{d}"][:])
            rec = const.tile([128, W], f32, tag=f"rec_{d}", name=f"rec_{d}")
            nc.vector.reciprocal(rec[:], degt[:])
            dv = const.tile([128, W], f32, tag=f"dinv_{d}", name=f"dinv_{d}")
            nc.scalar.activation(dv[:], rec[:], mybir.ActivationFunctionType.Sqrt)
            dinv[d] = dv

        # ---- phase A1: conv1 tables (both directions share xT loads) ----
        cw = meta["part"]["cw"]
        bounds = meta["td"]["bounds"]

        def emit_ag(d, l, q):
            nc.gpsimd.collective_compute(
                "AllGather", mybir.AluOpType.bypass, replica_groups=rg,
                ins=[ag_in[d, l][128 * int(cw[q]):128 * int(cw[q + 1]), :]],
                outs=[table[d, l][bounds[q]:bounds[q + 1], :]])

        nK = IN // 128
        for w in range(W):
            xts = []
            for kk in range(nK):
                t = xt_p.tile([128, 128], bf16, tag="xt", name=f"xt_{w}_{kk}")
                nc.sync.dma_start(t[:], ten["xT"][kk * 128:(kk + 1) * 128,
                                                 w * 128:(w + 1) * 128])
                xts.append(t)
            for d in ("td", "bu"):
                hps = hps_p.tile([128, HID], f32, tag="hps")
                for kk in range(nK):
                    nc.tensor.matmul(hps[:], xts[kk][:], Wt[d, 1][kk][:],
                                     start=(kk == 0), stop=(kk == nK - 1))
                hn = hn_p.tile([128, HID], bf16, tag="hn")
                nc.vector.tensor_scalar_mul(hn[:], hps[:], dinv[d][:, w:w + 1])
                nc.sync.dma_start(ag_in[d, 1][w * 128:(w + 1) * 128, :], hn[:])
            for q in range(NBLK):
                if w == int(cw[q + 1]) - 1:
                    emit_ag("td", 1, q)
                    emit_ag("bu", 1, q)

        # ---- edge phase for one conv ----
        def edge_phase(d, l):
            m = meta[d]
            first_mm = {}
            last_mm = {}
            # find last (sb_idx, group) per window for stop flags
            for sbi, sb in enumerate(m["struct"]):
                for i, w in enumerate(range(sb["w_lo"], sb["w_hi"])):
                    if sb["g_list"][i] > 0:
                        last_mm[w] = (sbi, int(sb["g_base"][i]) + int(sb["g_list"][i]) - 1)
            quad_tiles = {}
            def win_ap(w):
                q = w // 4
                if q not in quad_tiles:
                    qt = win_p.tile([128, 512], f32, tag="win",
                                    name=f"win_{d}{l}_{q}")
                    nc.tensor.matmul(qt[:], zrow[0:1, 0:128], zrow[0:1, 0:512],
                                     start=True, stop=False, skip_group_check=True)
                    quad_tiles[q] = qt
                return quad_tiles[q][:, (w % 4) * 128:(w % 4 + 1) * 128]
            for sbi, sb in enumerate(m["struct"]):
                G = sb["G"]
                if G == 0:
                    continue
                it = idx_p.tile([128, G * 8], i16, tag="idx")
                nc.sync.dma_start(it[:], ten[f"idx_{d}"][:, sb["off16"]:sb["off16"] + G * 8])
                dlt = dl_p.tile([128, G], bf16, tag="dl")
                nc.sync.dma_start(dlt[:], ten[f"dstloc_{d}"][:, sb["offG"]:sb["offG"] + G])
                gt = gat_p.tile([128, G, 128], bf16, tag="gat")
                blk = table[d, l][m["bounds"][sb["b"]]:m["bounds"][sb["b"] + 1], :]
                qn[0] += 1
                nc.gpsimd.dma_gather(gt[:], blk, it[:], num_idxs=G * 128,
                                     num_idxs_reg=G * 128, elem_size=HID,
                                     single_packet=False, queue_num=qn[0] % 4)
                oh = oh_p.tile([128, G * 128], bf16, tag="oh")
                nc.vector.tensor_tensor(
                    out=oh[:],
                    in0=dlt[:].rearrange("p (g o) -> p g o", o=1).to_broadcast([128, G, 128]),
                    in1=iota[:, :G * 128].rearrange("p (g f) -> p g f", f=128),
                    op=mybir.AluOpType.is_equal)
                for i, w in enumerate(range(sb["w_lo"], sb["w_hi"])):
                    gl = int(sb["g_list"][i])
                    if gl == 0:
                        continue
                    pt = win_ap(w)
                    gb = int(sb["g_base"][i])
                    for g in range(gb, gb + gl):
                        nc.tensor.matmul(
                            pt[:], oh[:, g * 128:(g + 1) * 128], gt[:, g, :],
                            start=False, stop=(last_mm[w] == (sbi, g)),
                            skip_group_check=True)
                # epilogues for completed supers: after last block of super
                if sb["b"] == NBLK - 1:
                    for w in range(sb["w_lo"], sb["w_hi"]):
                        epilogue(d, l, w, win_ap(w))
                    quad_tiles.clear()
                    yield sb["w_hi"]
                else:
                    yield None

        def epilogue(d, l, w, pt):
            hn = hn_p.tile([128, HID], bf16, tag="hn_ep")
            nc.sync.dma_start(hn[:], ag_in[d, l][w * 128:(w + 1) * 128, :])
            o1 = epi_p.tile([128, HID], f32, tag="o1")
            nc.vector.scalar_tensor_tensor(
                out=o1[:], in0=pt[:], scalar=dinv[d][:, w:w + 1], in1=bt[d, l][:],
                op0=mybir.AluOpType.mult, op1=mybir.AluOpType.add)
            o2 = epi_p.tile([128, HID], bf16, tag="o2")
            nc.vector.scalar_tensor_tensor(
                out=o2[:], in0=hn[:], scalar=dinv[d][:, w:w + 1], in1=o1[:],
                op0=mybir.AluOpType.mult, op1=mybir.AluOpType.add)
            if dbg and l == 1:
                mf = epi_p.tile([128, HID], f32, tag="mf")
                nc.vector.tensor_copy(mf[:], pt[:])
                nc.sync.dma_start(dbg_m[d][w * 128:(w + 1) * 128, :], mf[:])
            if l == 1:
                h1 = h1_p.tile([128, HID], bf16, tag="h1")
                nc.scalar.activation(h1[:], o2[:], mybir.ActivationFunctionType.Relu)
                if dbg:
                    h1f = epi_p.tile([128, HID], f32, tag="h1f")
                    nc.vector.tensor_copy(h1f[:], h1[:])
                    nc.sync.dma_start(dbg_h1[d][w * 128:(w + 1) * 128, :], h1f[:])
                tps = hps_p.tile([128, HID], bf16, tag="hps", name=f"tps_{d}_{w}")
                nc.tensor.transpose(tps[:], h1[:], ident[:])
                h1T = t_p.tile([128, HID], bf16, tag="h1T")
                nc.vector.tensor_copy(h1T[:], tps[:])
                h2 = hps_p.tile([128, HID], f32, tag="hps")
                nc.tensor.matmul(h2[:], h1T[:], Wt[d, 2][0][:], start=True, stop=True)
                hn2 = hn_p.tile([128, HID], bf16, tag="hn2")
                nc.vector.tensor_scalar_mul(hn2[:], h2[:], dinv[d][:, w:w + 1])
                nc.sync.dma_start(ag_in[d, 2][w * 128:(w + 1) * 128, :], hn2[:])
            else:
                po = po_p.tile([128, 128], bf16, tag="po")
                nc.vector.tensor_tensor(
                    out=po[:],
                    in0=batchloc[:, w:w + 1].to_broadcast([128, 128]),
                    in1=iota[:, :128],
                    op=mybir.AluOpType.is_equal)
                off = 0 if d == "td" else HID
                nc.tensor.matmul(pool_psum_t[:, off:off + HID], po[:], o2[:],
                                 start=False, stop=(w == W - 1),
                                 skip_group_check=True)

        qn = [0]

        def run_layer(l):
            gens = {"td": edge_phase("td", l), "bu": edge_phase("bu", l)}
            done = {"td": False, "bu": False}
            next_q = {"td": 0, "bu": 0}
            while not all(done.values()):
                for d in ("td", "bu"):
                    if done[d]:
                        continue
                    try:
                        res = next(gens[d])
                    except StopIteration:
                        done[d] = True
                        res = W
                    if l == 1 and res is not None:
                        while next_q[d] < NBLK and res >= int(cw[next_q[d] + 1]):
                            emit_ag(d, 2, next_q[d])
                            next_q[d] += 1

        run_layer(1)
        hps_cm.__exit__(None, None, None)
        pool_ps = stack.enter_context(tc.tile_pool(name="plps", bufs=1, space="PSUM"))
        pool_psum_t = pool_ps.tile([128, 2 * HID], f32, tag="pool", name="pool_psum_t")
        nc.tensor.matmul(pool_psum_t[:], zrow[0:1, 0:128], zrow[0:1, 0:2 * HID],
                         start=True, stop=False, skip_group_check=True)
        run_layer(2)

        outsb = outp.tile([128, 2 * HID], f32, tag="out")
        nc.vector.tensor_copy(outsb[:], pool_psum_t[:])
        nc.sync.dma_start(out_t[:], outsb[:])

    nc.compile()
    return nc


# =====================================================================
# Entry point
# =====================================================================

def _run(inputs, cfg, trace=False):
    from concourse import bass_utils
    x = np.asarray(inputs["x"], np.float32)
    edge_index = np.asarray(inputs["edge_index"])
    batch = np.asarray(inputs["batch"])
    Ws = [np.asarray(inputs[k], np.float32) for k in ("W_td1", "W_td2", "W_bu1", "W_bu2")]
    bs = [np.asarray(inputs[k], np.float32) for k in ("b_td1", "b_td2", "b_bu1", "b_bu2")]
    in_maps, meta = build_all_inputs(x, edge_index, batch, Ws, bs, cfg)
    nc = build_bass(meta)
    res = bass_utils.run_bass_kernel_spmd(
        nc, in_maps, core_ids=list(range(cfg["N_CORES"])), trace=trace)
    gpc = meta["part"]["gpc"]
    out = np.concatenate([res.results[c]["out"][:gpc] for c in range(cfg["N_CORES"])], axis=0)
    return out.astype(np.float32), res


def kernel(**inputs):
    out, _ = _run(inputs, FULL_CFG, trace=False)
    return out



# revision 18
# speedup vs baseline: 1.5531x; 1.5531x over previous
"""BiGCN (2-layer bidirectional GCN + global add pool) on 8 Trainium2 NeuronCores.

Strategy (hardcoded for the nn_BiGCN_graphcl problem shapes):
  - Nodes are sharded graph-aligned: core c owns graphs [128c, 128c+128) and
    their (contiguous, batch-sorted) node range, padded to a common NPC.
  - Per direction (td / bu), edges are assigned to the core owning their
    target node.  GCNConv is computed as
        out = dinv * (scatter_add(hn[src], dst) + hn) + b,   hn = dinv * (x @ W)
    so no per-edge scaling is needed on device.
  - The hn table ([8*NPC, 128] bf16) is AllGathered between layers; each core
    gathers rows for its edge shard with dma_gather (256B rows), builds a
    staircase one-hot with a DVE is_equal against an iota constant, and
    segment-sums on the TensorEngine into per-window (128-node) PSUM tiles.
  - The SPMD program is identical on all cores: all per-core variation lives
    in uploaded index/data tensors; run lengths are padded to the max across
    cores (pad slots gather row 0 of the block and carry dstloc=-1 so their
    one-hot column is zero).
  - Graph pooling is a second one-hot matmul into a [128 graphs, 128] PSUM
    tile; the host just concatenates the 8 per-core [128, 256] outputs.
"""

import math
import numpy as np
import ml_dtypes

BF16 = ml_dtypes.bfloat16

# ---------------------------------------------------------------- problem cfg
FULL_CFG = dict(
    N=100000, E=1600000, IN_FEATS=256, HIDDEN=128, OUT_FEATS=128,
    NUM_GRAPHS=1024, N_CORES=8, SW=8, NBLK=4,
)


def _round_up(x, m):
    return (x + m - 1) // m * m


# =====================================================================
# Host-side metadata construction
# =====================================================================

def build_partition(batch, cfg, deg_td=None, deg_bu=None):
    """Graph-aligned node partition. Returns dict with per-core node ranges.

    If degree arrays are given, each core's local node order is permuted so
    that per-window (128-node) degree sums cluster just under multiples of
    4*128 edges per (window, src-block) run, minimizing ceil-128 padding."""
    N, C, G = cfg["N"], cfg["N_CORES"], cfg["NUM_GRAPHS"]
    gpc = G // C  # graphs per core
    starts = np.searchsorted(batch, np.arange(0, G + 1, gpc))
    counts = np.diff(starts)
    NPC = max(128, _round_up(int(counts.max()), 128))
    W = NPC // 128
    node_core = np.searchsorted(starts[1:], np.arange(N), side="right")
    node_local = np.arange(N) - starts[node_core]

    if deg_td is not None:
        NBLK = cfg["NBLK"]
        MARGIN = 45 * NBLK  # leave room for cross-core/block-split variance
        for c in range(C):
            lo, hi = starts[c], starts[c + 1]
            cnt = hi - lo
            dt = deg_td[lo:hi].astype(np.int64)
            db = deg_bu[lo:hi].astype(np.int64)
            order = np.argsort(-(dt + db), kind="stable")
            tg_t = np.full(W, dt.sum() / W)
            tg_b = np.full(W, db.sum() / W)
            rem_t = tg_t.astype(np.float64).copy()
            rem_b = tg_b.astype(np.float64).copy()
            room = np.full(W, 128, np.int64)
            assign = np.empty(cnt, np.int64)
            for j in order:
                score = np.minimum(rem_t - dt[j], rem_b - db[j])
                score[room <= 0] = -np.inf
                w = int(np.argmax(score))
                assign[j] = w
                rem_t[w] -= dt[j]
                rem_b[w] -= db[j]
                room[w] -= 1
            # positions: window-major order
            slot_in_w = np.zeros(W, np.int64)
            newloc = np.empty(cnt, np.int64)
            for j in range(cnt):
                w = assign[j]
                newloc[j] = w * 128 + slot_in_w[w]
                slot_in_w[w] += 1
            node_local[lo:hi] = newloc

    # ---- chunk decomposition: 4 window-chunks, sized so per-(window, chunk)
    # gather runs land just under multiples of 128, and each chunk's block of
    # 8*128*w_q table rows stays within int16 index range. ----
    NBLK = cfg["NBLK"]
    mean_w = max(1.0, (deg_td.sum() + deg_bu.sum()) / (2.0 * C * W)) if deg_td is not None else 128.0
    wmax = min(W, (32767 // (128 * C)))

    def padfrac(wb):
        r = wb / W * mean_w  # mean edges per (window, this-chunk) run
        if r <= 0:
            return 0.0
        margin = 1.6 * np.sqrt(r) + 6
        gslots = 128 * np.ceil((r + margin) / 128)
        return (gslots - r) * 1.0

    best = None
    for w1 in range(1, wmax + 1):
        for w2 in range(w1, wmax + 1):
            for w3 in range(w2, wmax + 1):
                w4 = W - w1 - w2 - w3
                if w4 < w3 or w4 > wmax:
                    continue
                cost = padfrac(w1) + padfrac(w2) + padfrac(w3) + padfrac(w4)
                if best is None or cost < best[0]:
                    best = (cost, (w1, w2, w3, w4))
    ws = list(best[1]) if best else [W]
    # early chunks smaller -> earlier AG pipelining
    cw = np.concatenate([[0], np.cumsum(ws)])
    assert cw[-1] == W

    chunk_of_w = np.searchsorted(cw[1:], np.arange(W), side="right")
    q = chunk_of_w[np.minimum(node_local // 128, W - 1)]
    rpr = 128 * np.diff(cw)  # rows per rank per chunk
    base = np.concatenate([[0], np.cumsum(rpr * C)])
    table_row = base[q] + node_core * rpr[q] + (node_local - 128 * cw[q])
    bounds = [int(b) for b in base]
    return dict(starts=starts, counts=counts, NPC=NPC, gpc=gpc,
                node_core=node_core.astype(np.int64),
                node_local=node_local.astype(np.int64),
                table_row=table_row.astype(np.int64),
                cw=cw, bounds=bounds)


def build_direction_meta(gather_nodes, target_nodes, part, cfg):
    """Build per-core gather index / dstloc arrays and the uniform group
    structure for one edge direction.

    gather_nodes[e]: node whose table row is gathered for edge e.
    target_nodes[e]: node receiving the contribution.
    """
    N, C = cfg["N"], cfg["N_CORES"]
    SW, NBLK = cfg["SW"], cfg["NBLK"]
    NPC = part["NPC"]
    W = NPC // 128
    NS = (W + SW - 1) // SW
    R = C * NPC

    deg = np.bincount(target_nodes, minlength=N).astype(np.float64) + 1.0

    bounds = part["bounds"]
    assert len(bounds) == NBLK + 1
    assert all(bounds[i + 1] - bounds[i] <= 32767 for i in range(NBLK))
    bounds_arr = np.array(bounds[1:-1])

    tr_g = part["table_row"][gather_nodes]
    t_core = part["node_core"][target_nodes]
    t_local = part["node_local"][target_nodes]
    lw = t_local // 128          # window
    dloc = t_local % 128         # position within window
    blk = np.searchsorted(bounds_arr, tr_g, side="right")
    idxv = tr_g - np.array(bounds[:-1])[blk]
    sup = lw // SW

    # per (core, s, b, w) counts -> uniform G
    keyW = (sup * NBLK + blk) * W + lw  # key within a core
    nkeys = NS * NBLK * W
    counts = np.zeros((C, nkeys), np.int64)
    for c in range(C):
        m = t_core == c
        counts[c] = np.bincount(keyW[m], minlength=nkeys)
    max_counts = counts.max(axis=0).reshape(NS, NBLK, W)

    G = np.ceil(max_counts / 128).astype(np.int64)  # groups per (s,b,w)
    # ensure every window has at least one group (psum must be written)
    for s in range(NS):
        w_lo, w_hi = s * SW, min((s + 1) * SW, W)
        for w in range(w_lo, w_hi):
            if G[s, :, w].sum() == 0:
                G[s, 0, w] = 1
        G[s, :, :w_lo] = 0
        G[s, :, w_hi:] = 0

    # structure: per (s,b): window col bases, totals
    struct = []
    for s in range(NS):
        w_lo, w_hi = s * SW, min((s + 1) * SW, W)
        for b in range(NBLK):
            g_list = G[s, b, w_lo:w_hi]
            base = np.concatenate([[0], np.cumsum(g_list)])
            struct.append(dict(s=s, b=b, w_lo=w_lo, w_hi=w_hi,
                               g_list=g_list, g_base=base,
                               G=int(g_list.sum())))
    # global column offsets
    offG = 0
    off16 = 0
    for sb in struct:
        sb["offG"] = offG
        sb["off16"] = off16
        offG += sb["G"]
        off16 += sb["G"] * 8  # 128 slots / 16
    CG = offG
    Gmax = max((sb["G"] for sb in struct), default=1)

    # per-edge slot assignment (per core)
    idx_all = np.zeros((C, 128, CG * 8), np.int16)
    dloc_all = np.full((C, 128, CG), -1.0, BF16)
    # precompute slot base for each (s,b,w): global slot start
    slot_base = np.zeros((NS, NBLK, W), np.int64)
    for sb in struct:
        s, b = sb["s"], sb["b"]
        for i, w in enumerate(range(sb["w_lo"], sb["w_hi"])):
            slot_base[s, b, w] = (sb["offG"] + sb["g_base"][i]) * 128

    for c in range(C):
        m = t_core == c
        k = keyW[m]
        order = np.argsort(k, kind="stable")
        ks = k[order]
        # rank within each run
        run_start = np.searchsorted(ks, np.arange(nkeys))
        rank = np.arange(len(ks)) - run_start[ks]
        sb_s = ks // (NBLK * W)
        sb_b = (ks // W) % NBLK
        sb_w = ks % W
        slot = slot_base[sb_s, sb_b, sb_w] + rank
        iv = idxv[m][order]
        dv = dloc[m][order]
        # idx wrapped layout: slot j -> (j%16, j//16), replicated x8
        prow = slot % 16
        pcol = slot // 16
        tmp = np.zeros((16, CG * 8), np.int16)
        tmp[prow, pcol] = iv.astype(np.int16)
        idx_all[c] = np.tile(tmp, (8, 1))
        dloc_all[c, slot % 128, slot // 128] = dv.astype(BF16)

    return dict(deg=deg, struct=struct, CG=CG, Gmax=Gmax, NS=NS, W=W,
                bounds=bounds, idx_all=idx_all, dloc_all=dloc_all)


def build_pool_meta(src, dst, batch, part, td_deg, bu_deg, cfg):
    """Layer-2-as-pooled-matmul coefficients.

    out_graph[g] = sum_v M[row(v), g] * hn2[v] + n_g * b2, where hn2 is the
    (AllGathered) dinv*(h1@W2) table in chunked table-row order.  M folds the
    edge aggregation (target-side dinv summed per source node and graph) and
    the self-loop diagonal."""
    C, N, G = cfg["N_CORES"], cfg["N"], cfg["NUM_GRAPHS"]
    gpc = G // C
    R = part["bounds"][-1]
    tr = part["table_row"]
    node_core = part["node_core"]
    batch = np.asarray(batch)

    dinv_td = 1.0 / np.sqrt(td_deg)          # [N] float64
    dinv_bu = 1.0 / np.sqrt(bu_deg)

    M_td = np.zeros((C, R, 128), np.float32)
    M_bu = np.zeros((C, R, 128), np.float32)
    # td: value row src, target dst -> coeff dinv_td[dst] into (core(dst), g(dst))
    c_t = node_core[dst]
    np.add.at(M_td, (c_t, tr[src], batch[dst] - c_t * gpc), dinv_td[dst].astype(np.float32))
    # bu: value row dst, target src -> coeff dinv_bu[src] into (core(src), g(src))
    c_s = node_core[src]
    np.add.at(M_bu, (c_s, tr[dst], batch[src] - c_s * gpc), dinv_bu[src].astype(np.float32))
    # self-loop diagonals: M[row(v), g(v)] += dinv[v] on the core owning v
    allv = np.arange(N)
    c_v = node_core[allv]
    np.add.at(M_td, (c_v, tr[allv], batch[allv] - c_v * gpc), dinv_td[allv].astype(np.float32))
    np.add.at(M_bu, (c_v, tr[allv], batch[allv] - c_v * gpc), dinv_bu[allv].astype(np.float32))
    n_g = np.bincount(batch, minlength=G).astype(np.float32)  # nodes per graph
    return dict(M_td=M_td.astype(BF16), M_bu=M_bu.astype(BF16), n_g=n_g, R=R)


def pool_batches(part, bw=32):
    """Window batches for the pooled-matmul phase, aligned to AG chunks."""
    bounds = part["bounds"]
    batches = []
    for q in range(len(bounds) - 1):
        u0 = bounds[q] // 128
        nwq = (bounds[q + 1] - bounds[q]) // 128
        for s in range(0, nwq, bw):
            batches.append((q, u0 + s, min(bw, nwq - s)))
    return batches


def pack_M(M, batches):
    """Repack [R, G] so each batch's block reads contiguously per partition:
    row p*nw+j holds window (u0+j) partition p."""
    out = np.empty_like(M)
    for (q, u0, nw) in batches:
        blk = M[u0 * 128:(u0 + nw) * 128].reshape(nw, 128, -1)
        out[u0 * 128:(u0 + nw) * 128] = blk.transpose(1, 0, 2).reshape(nw * 128, -1)
    return out


def build_all_inputs(x, edge_index, batch, Ws, bs, cfg):
    """Produce per-core in_maps plus structural metadata."""
    C = cfg["N_CORES"]
    N = cfg["N"]
    src = np.asarray(edge_index[0])
    dst = np.asarray(edge_index[1])
    part = build_partition(batch, cfg,
                           deg_td=np.bincount(dst, minlength=N),
                           deg_bu=np.bincount(src, minlength=N))
    NPC = part["NPC"]
    W = NPC // 128

    td = build_direction_meta(src, dst, part, cfg)   # gather src row, scatter to dst
    bu = build_direction_meta(dst, src, part, cfg)   # reversed
    pool = build_pool_meta(src, dst, batch, part, td["deg"], bu["deg"], cfg)
    batches = pool_batches(part)
    pool["M_td"] = np.stack([pack_M(pool["M_td"][c], batches) for c in range(C)])
    pool["M_bu"] = np.stack([pack_M(pool["M_bu"][c], batches) for c in range(C)])

    Gmax = max(td["Gmax"], bu["Gmax"])
    iota_rep = np.tile(np.arange(128, dtype=np.float32), Gmax)[None, :].repeat(128, 0).astype(BF16)

    # per-core tensors
    in_maps = []
    xT_full = np.ascontiguousarray(np.asarray(x).T)  # [IN, N]
    batch_np = np.asarray(batch)
    ngb2 = np.concatenate([np.outer(pool["n_g"], bs[1]),
                           np.outer(pool["n_g"], bs[3])], axis=1).astype(np.float32)
    for c in range(C):
        lo, hi = part["starts"][c], part["starts"][c + 1]
        cnt = hi - lo
        li = part["node_local"][lo:hi]
        xT = np.zeros((cfg["IN_FEATS"], NPC), BF16)
        xT[:, li] = xT_full[:, lo:hi].astype(BF16)
        deg_t = np.ones((128, W), np.float32)
        deg_b = np.ones((128, W), np.float32)
        deg_t[li % 128, li // 128] = td["deg"][lo:hi].astype(np.float32)
        deg_b[li % 128, li // 128] = bu["deg"][lo:hi].astype(np.float32)
        im = dict(
            xT=xT, ident=np.eye(128, dtype=BF16),
            deg_td=deg_t, deg_bu=deg_b, iota_rep=iota_rep,
            M_td=pool["M_td"][c], M_bu=pool["M_bu"][c],
            ngb2=ngb2[c * part["gpc"]:(c + 1) * part["gpc"]],
            idx_td=td["idx_all"][c], idx_bu=bu["idx_all"][c],
            dstloc_td=td["dloc_all"][c], dstloc_bu=bu["dloc_all"][c],
            W_td1=Ws[0].astype(BF16), W_bu1=Ws[2].astype(BF16),
            W_td2=Ws[1].astype(BF16), W_bu2=Ws[3].astype(BF16),
            b_td1=np.tile(bs[0][None, :], (128, 1)).astype(np.float32),
            b_td2=np.tile(bs[1][None, :], (128, 1)).astype(np.float32),
            b_bu1=np.tile(bs[2][None, :], (128, 1)).astype(np.float32),
            b_bu2=np.tile(bs[3][None, :], (128, 1)).astype(np.float32),
        )
        in_maps.append(im)
    meta = dict(part=part, td=td, bu=bu, Gmax=Gmax, NPC=NPC, W=W, cfg=cfg,
                R=pool["R"], batches=batches)
    return in_maps, meta


# =====================================================================
# Bass program
# =====================================================================

def build_bass(meta):
    import concourse.bacc as bacc
    import concourse.mybir as mybir
    import concourse.tile as tile

    cfg = meta["cfg"]
    C = cfg["N_CORES"]
    NPC, W, Gmax = meta["NPC"], meta["W"], meta["Gmax"]
    IN, HID = cfg["IN_FEATS"], cfg["HIDDEN"]
    NBLK = cfg["NBLK"]
    f32, bf16, i16 = mybir.dt.float32, mybir.dt.bfloat16, mybir.dt.int16

    nc = bacc.Bacc("TRN2", target_bir_lowering=False, debug=False, num_devices=C,
                   num_swdge_queues=4)

    # ---- I/O ----
    ten = {}
    def inp(name, shape, dt):
        ten[name] = nc.dram_tensor(name, shape, dt, kind="ExternalInput")
        return ten[name]

    inp("xT", [IN, NPC], bf16)
    inp("deg_td", [128, W], f32); inp("deg_bu", [128, W], f32)
    inp("iota_rep", [128, Gmax * 128], bf16)
    inp("ident", [128, 128], bf16)
    inp("ngb2", [128, 2 * HID], f32)
    R = meta["R"]
    for d in ("td", "bu"):
        m = meta[d]
        inp(f"idx_{d}", [128, m["CG"] * 8], i16)
        inp(f"dstloc_{d}", [128, m["CG"]], bf16)
        inp(f"M_{d}", [R, 128], bf16)
        inp(f"W_{d}1", [IN, HID], bf16)
        inp(f"W_{d}2", [HID, HID], bf16)
        inp(f"b_{d}1", [128, HID], f32)
        inp(f"b_{d}2", [128, HID], f32)
    out_t = nc.dram_tensor("out", [128, 2 * HID], f32, kind="ExternalOutput")
    dbg = meta.get("dbg")
    if dbg:
        dbg_h1 = {d: nc.dram_tensor(f"dbg_h1_{d}", [NPC, HID], f32, kind="ExternalOutput")
                  for d in ("td", "bu")}
        dbg_m = {d: nc.dram_tensor(f"dbg_m_{d}", [NPC, HID], f32, kind="ExternalOutput")
                 for d in ("td", "bu")}

    # internal DRAM: AG inputs + tables
    ag_in, table = {}, {}
    for d in ("td", "bu"):
        for l in (1, 2):
            ag_in[d, l] = nc.dram_tensor(f"agin_{d}{l}", [NPC, HID], bf16, kind="Internal")
            table[d, l] = nc.dram_tensor(f"table_{d}{l}", [C * NPC, HID], bf16,
                                         kind="Internal", addr_space="Shared")

    rg = [list(range(C))]

    from contextlib import ExitStack
    with tile.TileContext(nc) as tc, ExitStack() as stack:
        def pool(name, bufs, space="SBUF"):
            return stack.enter_context(tc.tile_pool(name=name, bufs=bufs, space=space))

        const = pool("const", 1)
        xt_p = pool("xt", 6)
        hn_p = pool("hn", 4)                 # hn tiles to DRAM
        idx_p = pool("idx", 4)
        dl_p = pool("dl", 4)
        gat_p = pool("gat", 5)               # gathered edge tiles
        oh_p = pool("oh", 3)                 # one-hot tiles
        win_p = pool("win", 5, "PSUM")       # window psum, 4 windows/bank
        epi_p = pool("epi", 6)               # epilogue sbuf tiles
        h1_p = pool("h1", 4)
        t_p = pool("tt", 4)                  # transposes
        mb_p = pool("mb", 2)                 # pooled-matmul M batches
        tb_p = pool("tb", 2)                 # pooled-matmul hn2 batches
        outp = pool("outp", 1)
        hps_p = pool("hps", 2, "PSUM")
        pool_ps = pool("plps", 1, "PSUM")

        # ---- constants in SBUF ----
        iota = const.tile([128, Gmax * 128], bf16, tag="iota")
        nc.sync.dma_start(iota[:], ten["iota_rep"][:])
        Wt = {}
        for d in ("td", "bu"):
            for l, k in ((1, IN), (2, HID)):
                chunks = []
                for kk in range(k // 128):
                    t = const.tile([128, HID], bf16, tag=f"W_{d}{l}_{kk}", name=f"W_{d}{l}_{kk}")
                    nc.sync.dma_start(t[:], ten[f"W_{d}{l}"][kk * 128:(kk + 1) * 128, :])
                    chunks.append(t)
                Wt[d, l] = chunks
        bt = {}
        for d in ("td", "bu"):
            for l in (1, 2):
                t = const.tile([128, HID], f32, tag=f"b_{d}{l}", name=f"bt_{d}{l}")
                nc.sync.dma_start(t[:], ten[f"b_{d}{l}"][:])
                bt[d, l] = t
        zrow = const.tile([1, 512], bf16, tag="zrow")
        nc.gpsimd.memset(zrow[:], 0.0)
        ident = const.tile([128, 128], bf16, tag="ident")
        nc.sync.dma_start(ident[:], ten["ident"][:])
        ngb2_t = const.tile([128, 2 * HID], f32, tag="ngb2")
        nc.sync.dma_start(ngb2_t[:], ten["ngb2"][:])

        dinv = {}
        for d in ("td", "bu"):
            degt = const.tile([128, W], f32, tag=f"deg_{d}", name=f"degt_{d}")
            nc.sync.dma_start(degt[:], ten[f"deg_{d}"][:])
            rec = const.tile([128, W], f32, tag=f"rec_{d}", name=f"rec_{d}")
            nc.vector.reciprocal(rec[:], degt[:])
            dv = const.tile([128, W], f32, tag=f"dinv_{d}", name=f"dinv_{d}")
            nc.scalar.activation(dv[:], rec[:], mybir.ActivationFunctionType.Sqrt)
            dinv[d] = dv

        # ---- phase A1: conv1 tables (both directions share xT loads) ----
        cw = meta["part"]["cw"]
        bounds = meta["td"]["bounds"]

        def emit_ag(d, l, q):
            nc.gpsimd.collective_compute(
                "AllGather", mybir.AluOpType.bypass, replica_groups=rg,
                ins=[ag_in[d, l][128 * int(cw[q]):128 * int(cw[q + 1]), :]],
                outs=[table[d, l][bounds[q]:bounds[q + 1], :]])

        nK = IN // 128
        for w in range(W):
            xts = []
            for kk in range(nK):
                t = xt_p.tile([128, 128], bf16, tag="xt", name=f"xt_{w}_{kk}")
                nc.sync.dma_start(t[:], ten["xT"][kk * 128:(kk + 1) * 128,
                                                 w * 128:(w + 1) * 128])
                xts.append(t)
            for d in ("td", "bu"):
                hps = hps_p.tile([128, HID], f32, tag="hps")
                for kk in range(nK):
                    nc.tensor.matmul(hps[:], xts[kk][:], Wt[d, 1][kk][:],
                                     start=(kk == 0), stop=(kk == nK - 1))
                hn = hn_p.tile([128, HID], bf16, tag="hn")
                nc.vector.tensor_scalar_mul(hn[:], hps[:], dinv[d][:, w:w + 1])
                nc.sync.dma_start(ag_in[d, 1][w * 128:(w + 1) * 128, :], hn[:])
            for q in range(NBLK):
                if w == int(cw[q + 1]) - 1:
                    emit_ag("td", 1, q)
                    emit_ag("bu", 1, q)

        # ---- edge phase for one conv ----
        def edge_phase(d, l):
            m = meta[d]
            first_mm = {}
            last_mm = {}
            # find last (sb_idx, group) per window for stop flags
            for sbi, sb in enumerate(m["struct"]):
                for i, w in enumerate(range(sb["w_lo"], sb["w_hi"])):
                    if sb["g_list"][i] > 0:
                        last_mm[w] = (sbi, int(sb["g_base"][i]) + int(sb["g_list"][i]) - 1)
            quad_tiles = {}
            def win_ap(w):
                q = w // 4
                if q not in quad_tiles:
                    qt = win_p.tile([128, 512], f32, tag="win",
                                    name=f"win_{d}{l}_{q}")
                    nc.tensor.matmul(qt[:], zrow[0:1, 0:128], zrow[0:1, 0:512],
                                     start=True, stop=False, skip_group_check=True)
                    quad_tiles[q] = qt
                return quad_tiles[q][:, (w % 4) * 128:(w % 4 + 1) * 128]
            for sbi, sb in enumerate(m["struct"]):
                G = sb["G"]
                if G == 0:
                    continue
                it = idx_p.tile([128, G * 8], i16, tag="idx")
                nc.sync.dma_start(it[:], ten[f"idx_{d}"][:, sb["off16"]:sb["off16"] + G * 8])
                dlt = dl_p.tile([128, G], bf16, tag="dl")
                nc.sync.dma_start(dlt[:], ten[f"dstloc_{d}"][:, sb["offG"]:sb["offG"] + G])
                gt = gat_p.tile([128, G, 128], bf16, tag="gat")
                blk = table[d, l][m["bounds"][sb["b"]]:m["bounds"][sb["b"] + 1], :]
                qn[0] += 1
                nc.gpsimd.dma_gather(gt[:], blk, it[:], num_idxs=G * 128,
                                     num_idxs_reg=G * 128, elem_size=HID,
                                     single_packet=False, queue_num=qn[0] % 4)
                oh = oh_p.tile([128, G * 128], bf16, tag="oh")
                nc.vector.tensor_tensor(
                    out=oh[:],
                    in0=dlt[:].rearrange("p (g o) -> p g o", o=1).to_broadcast([128, G, 128]),
                    in1=iota[:, :G * 128].rearrange("p (g f) -> p g f", f=128),
                    op=mybir.AluOpType.is_equal)
                for i, w in enumerate(range(sb["w_lo"], sb["w_hi"])):
                    gl = int(sb["g_list"][i])
                    if gl == 0:
                        continue
                    pt = win_ap(w)
                    gb = int(sb["g_base"][i])
                    for g in range(gb, gb + gl):
                        nc.tensor.matmul(
                            pt[:], oh[:, g * 128:(g + 1) * 128], gt[:, g, :],
                            start=False, stop=(last_mm[w] == (sbi, g)),
                            skip_group_check=True)
                # epilogues for completed supers: after last block of super
                if sb["b"] == NBLK - 1:
                    for w in range(sb["w_lo"], sb["w_hi"]):
                        epilogue(d, l, w, win_ap(w))
                    quad_tiles.clear()
                    yield sb["w_hi"]
                else:
                    yield None

        def epilogue(d, l, w, pt):
            hn = hn_p.tile([128, HID], bf16, tag="hn_ep")
            nc.sync.dma_start(hn[:], ag_in[d, l][w * 128:(w + 1) * 128, :])
            o1 = epi_p.tile([128, HID], f32, tag="o1")
            nc.vector.scalar_tensor_tensor(
                out=o1[:], in0=pt[:], scalar=dinv[d][:, w:w + 1], in1=bt[d, l][:],
                op0=mybir.AluOpType.mult, op1=mybir.AluOpType.add)
            o2 = epi_p.tile([128, HID], bf16, tag="o2")
            nc.vector.scalar_tensor_tensor(
                out=o2[:], in0=hn[:], scalar=dinv[d][:, w:w + 1], in1=o1[:],
                op0=mybir.AluOpType.mult, op1=mybir.AluOpType.add)
            if dbg and l == 1:
                mf = epi_p.tile([128, HID], f32, tag="mf")
                nc.vector.tensor_copy(mf[:], pt[:])
                nc.sync.dma_start(dbg_m[d][w * 128:(w + 1) * 128, :], mf[:])
            h1 = h1_p.tile([128, HID], bf16, tag="h1")
            nc.scalar.activation(h1[:], o2[:], mybir.ActivationFunctionType.Relu)
            if dbg:
                h1f = epi_p.tile([128, HID], f32, tag="h1f")
                nc.vector.tensor_copy(h1f[:], h1[:])
                nc.sync.dma_start(dbg_h1[d][w * 128:(w + 1) * 128, :], h1f[:])
            tps = hps_p.tile([128, HID], bf16, tag="hps", name=f"tps_{d}_{w}")
            nc.tensor.transpose(tps[:], h1[:], ident[:])
            h1T = t_p.tile([128, HID], bf16, tag="h1T")
            nc.vector.tensor_copy(h1T[:], tps[:])
            h2 = hps_p.tile([128, HID], f32, tag="hps")
            nc.tensor.matmul(h2[:], h1T[:], Wt[d, 2][0][:], start=True, stop=True)
            hn2 = hn_p.tile([128, HID], bf16, tag="hn2")
            nc.vector.tensor_scalar_mul(hn2[:], h2[:], dinv[d][:, w:w + 1])
            nc.sync.dma_start(ag_in[d, 2][w * 128:(w + 1) * 128, :], hn2[:])

        # ---- layer-2 pooled matmul: out[g] += M_u^T @ hn2_u per table window ----
        batches = meta["batches"]
        last_uq = {}   # d -> (q, u0, nw) of final batch
        for d in ("td", "bu"):
            last_uq[d] = batches[-1]

        def emit_pool(d, q):
            off = 0 if d == "td" else HID
            for (bq, u0, nw) in batches:
                if bq != q:
                    continue
                mt = mb_p.tile([128, nw, HID], bf16, tag="mb")
                nc.scalar.dma_start(
                    mt[:], ten[f"M_{d}"][u0 * 128:(u0 + nw) * 128, :]
                    .rearrange("(p j) g -> p j g", p=128))
                ht = tb_p.tile([128, nw, HID], bf16, tag="tb")
                nc.scalar.dma_start(
                    ht[:], table[d, 2][u0 * 128:(u0 + nw) * 128, :]
                    .rearrange("(j p) f -> p j f", p=128))
                for j in range(nw):
                    is_last = (bq, u0, nw) == last_uq[d] and j == nw - 1
                    nc.tensor.matmul(
                        pool_psum_t[:, off:off + HID],
                        mt[:, j, :], ht[:, j, :],
                        start=False, stop=is_last, skip_group_check=True)

        qn = [0]

        pool_psum_t = pool_ps.tile([128, 2 * HID], f32, tag="pool", name="pool_psum_t")
        nc.tensor.matmul(pool_psum_t[:], zrow[0:1, 0:128], zrow[0:1, 0:2 * HID],
                         start=True, stop=False, skip_group_check=True)

        def run_layer(l):
            gens = {"td": edge_phase("td", l), "bu": edge_phase("bu", l)}
            done = {"td": False, "bu": False}
            next_q = {"td": 0, "bu": 0}
            while not all(done.values()):
                for d in ("td", "bu"):
                    if done[d]:
                        continue
                    try:
                        res = next(gens[d])
                    except StopIteration:
                        done[d] = True
                        res = W
                    if l == 1 and res is not None:
                        while next_q[d] < NBLK and res >= int(cw[next_q[d] + 1]):
                            emit_ag(d, 2, next_q[d])
                            emit_pool(d, next_q[d])
                            next_q[d] += 1

        run_layer(1)

        outsb = outp.tile([128, 2 * HID], f32, tag="out")
        nc.vector.tensor_tensor(out=outsb[:], in0=pool_psum_t[:], in1=ngb2_t[:],
                                op=mybir.AluOpType.add)
        nc.sync.dma_start(out_t[:], outsb[:])

    nc.compile()
    return nc


# =====================================================================
# Entry point
# =====================================================================

def _run(inputs, cfg, trace=False):
    from concourse import bass_utils
    x = np.asarray(inputs["x"], np.float32)
    edge_index = np.asarray(inputs["edge_index"])
    batch = np.asarray(inputs["batch"])
    Ws = [np.asarray(inputs[k], np.float32) for k in ("W_td1", "W_td2", "W_bu1", "W_bu2")]
    bs = [np.asarray(inputs[k], np.float32) for k in ("b_td1", "b_td2", "b_bu1", "b_bu2")]
    in_maps, meta = build_all_inputs(x, edge_index, batch, Ws, bs, cfg)
    nc = build_bass(meta)
    res = bass_utils.run_bass_kernel_spmd(
        nc, in_maps, core_ids=list(range(cfg["N_CORES"])), trace=trace)
    gpc = meta["part"]["gpc"]
    out = np.concatenate([res.results[c]["out"][:gpc] for c in range(cfg["N_CORES"])], axis=0)
    return out.astype(np.float32), res


def kernel(**inputs):
    out, _ = _run(inputs, FULL_CFG, trace=False)
    return out



# revision 24
# speedup vs baseline: 1.6104x; 1.0369x over previous
"""BiGCN (2-layer bidirectional GCN + global add pool) on 8 Trainium2 NeuronCores.

Strategy (hardcoded for the nn_BiGCN_graphcl problem shapes):
  - Nodes are sharded graph-aligned: core c owns graphs [128c, 128c+128) and
    their (contiguous, batch-sorted) node range, padded to a common NPC.
  - Per direction (td / bu), edges are assigned to the core owning their
    target node.  GCNConv is computed as
        out = dinv * (scatter_add(hn[src], dst) + hn) + b,   hn = dinv * (x @ W)
    so no per-edge scaling is needed on device.
  - The hn table ([8*NPC, 128] bf16) is AllGathered between layers; each core
    gathers rows for its edge shard with dma_gather (256B rows), builds a
    staircase one-hot with a DVE is_equal against an iota constant, and
    segment-sums on the TensorEngine into per-window (128-node) PSUM tiles.
  - The SPMD program is identical on all cores: all per-core variation lives
    in uploaded index/data tensors; run lengths are padded to the max across
    cores (pad slots gather row 0 of the block and carry dstloc=-1 so their
    one-hot column is zero).
  - Graph pooling is a second one-hot matmul into a [128 graphs, 128] PSUM
    tile; the host just concatenates the 8 per-core [128, 256] outputs.
"""

import math
import numpy as np
import ml_dtypes

BF16 = ml_dtypes.bfloat16

# ---------------------------------------------------------------- problem cfg
FULL_CFG = dict(
    N=100000, E=1600000, IN_FEATS=256, HIDDEN=128, OUT_FEATS=128,
    NUM_GRAPHS=1024, N_CORES=8, SW=8, NBLK=4,
)


def _round_up(x, m):
    return (x + m - 1) // m * m


# =====================================================================
# Host-side metadata construction
# =====================================================================

def build_partition(batch, cfg, deg_td=None, deg_bu=None):
    """Graph-aligned node partition. Returns dict with per-core node ranges.

    If degree arrays are given, each core's local node order is permuted so
    that per-window (128-node) degree sums cluster just under multiples of
    4*128 edges per (window, src-block) run, minimizing ceil-128 padding."""
    N, C, G = cfg["N"], cfg["N_CORES"], cfg["NUM_GRAPHS"]
    gpc = G // C  # graphs per core
    starts = np.searchsorted(batch, np.arange(0, G + 1, gpc))
    counts = np.diff(starts)
    NPC = max(128, _round_up(int(counts.max()), 128))
    W = NPC // 128
    node_core = np.searchsorted(starts[1:], np.arange(N), side="right")
    node_local = np.arange(N) - starts[node_core]

    if deg_td is not None:
        NBLK = cfg["NBLK"]
        MARGIN = 45 * NBLK  # leave room for cross-core/block-split variance
        for c in range(C):
            lo, hi = starts[c], starts[c + 1]
            cnt = hi - lo
            dt = deg_td[lo:hi].astype(np.int64)
            db = deg_bu[lo:hi].astype(np.int64)
            order = np.argsort(-(dt + db), kind="stable")
            tg_t = np.full(W, dt.sum() / W)
            tg_b = np.full(W, db.sum() / W)
            rem_t = tg_t.astype(np.float64).copy()
            rem_b = tg_b.astype(np.float64).copy()
            room = np.full(W, 128, np.int64)
            assign = np.empty(cnt, np.int64)
            for j in order:
                score = np.minimum(rem_t - dt[j], rem_b - db[j])
                score[room <= 0] = -np.inf
                w = int(np.argmax(score))
                assign[j] = w
                rem_t[w] -= dt[j]
                rem_b[w] -= db[j]
                room[w] -= 1
            # positions: window-major order
            slot_in_w = np.zeros(W, np.int64)
            newloc = np.empty(cnt, np.int64)
            for j in range(cnt):
                w = assign[j]
                newloc[j] = w * 128 + slot_in_w[w]
                slot_in_w[w] += 1
            node_local[lo:hi] = newloc

    # ---- chunk decomposition: 4 window-chunks, sized so per-(window, chunk)
    # gather runs land just under multiples of 128, and each chunk's block of
    # 8*128*w_q table rows stays within int16 index range. ----
    NBLK = cfg["NBLK"]
    mean_w = max(1.0, (deg_td.sum() + deg_bu.sum()) / (2.0 * C * W)) if deg_td is not None else 128.0
    wmax = min(W, (32767 // (128 * C)))

    def padfrac(wb):
        r = wb / W * mean_w  # mean edges per (window, this-chunk) run
        if r <= 0:
            return 0.0
        margin = 1.6 * np.sqrt(r) + 6
        gslots = 128 * np.ceil((r + margin) / 128)
        return (gslots - r) * 1.0

    best = None
    for w1 in range(1, wmax + 1):
        for w2 in range(w1, wmax + 1):
            for w3 in range(w2, wmax + 1):
                w4 = W - w1 - w2 - w3
                if w4 < w3 or w4 > wmax:
                    continue
                cost = padfrac(w1) + padfrac(w2) + padfrac(w3) + padfrac(w4)
                if best is None or cost < best[0]:
                    best = (cost, (w1, w2, w3, w4))
    ws = sorted(best[1], reverse=True) if best else [W]
    # big chunks first: their AG starts earliest, and the LAST chunk (whose
    # pooled-matmul work forms the serial tail) is smallest
    cw = np.concatenate([[0], np.cumsum(ws)])
    assert cw[-1] == W

    chunk_of_w = np.searchsorted(cw[1:], np.arange(W), side="right")
    q = chunk_of_w[np.minimum(node_local // 128, W - 1)]
    rpr = 128 * np.diff(cw)  # rows per rank per chunk
    base = np.concatenate([[0], np.cumsum(rpr * C)])
    table_row = base[q] + node_core * rpr[q] + (node_local - 128 * cw[q])
    bounds = [int(b) for b in base]
    return dict(starts=starts, counts=counts, NPC=NPC, gpc=gpc,
                node_core=node_core.astype(np.int64),
                node_local=node_local.astype(np.int64),
                table_row=table_row.astype(np.int64),
                cw=cw, bounds=bounds)


def build_direction_meta(gather_nodes, target_nodes, part, cfg):
    """Build per-core gather index / dstloc arrays and the uniform group
    structure for one edge direction.

    gather_nodes[e]: node whose table row is gathered for edge e.
    target_nodes[e]: node receiving the contribution.
    """
    N, C = cfg["N"], cfg["N_CORES"]
    SW, NBLK = cfg["SW"], cfg["NBLK"]
    NPC = part["NPC"]
    W = NPC // 128
    NS = (W + SW - 1) // SW
    R = C * NPC

    deg = np.bincount(target_nodes, minlength=N).astype(np.float64) + 1.0

    bounds = part["bounds"]
    assert len(bounds) == NBLK + 1
    assert all(bounds[i + 1] - bounds[i] <= 32767 for i in range(NBLK))
    bounds_arr = np.array(bounds[1:-1])

    tr_g = part["table_row"][gather_nodes]
    t_core = part["node_core"][target_nodes]
    t_local = part["node_local"][target_nodes]
    lw = t_local // 128          # window
    dloc = t_local % 128         # position within window
    blk = np.searchsorted(bounds_arr, tr_g, side="right")
    idxv = tr_g - np.array(bounds[:-1])[blk]
    sup = lw // SW

    # per (core, s, b, w) counts -> uniform G
    keyW = (sup * NBLK + blk) * W + lw  # key within a core
    nkeys = NS * NBLK * W
    counts = np.zeros((C, nkeys), np.int64)
    for c in range(C):
        m = t_core == c
        counts[c] = np.bincount(keyW[m], minlength=nkeys)
    max_counts = counts.max(axis=0).reshape(NS, NBLK, W)

    G = np.ceil(max_counts / 128).astype(np.int64)  # groups per (s,b,w)
    # ensure every window has at least one group (psum must be written)
    for s in range(NS):
        w_lo, w_hi = s * SW, min((s + 1) * SW, W)
        for w in range(w_lo, w_hi):
            if G[s, :, w].sum() == 0:
                G[s, 0, w] = 1
        G[s, :, :w_lo] = 0
        G[s, :, w_hi:] = 0

    # structure: per (s,b): window col bases, totals
    struct = []
    for s in range(NS):
        w_lo, w_hi = s * SW, min((s + 1) * SW, W)
        for b in range(NBLK):
            g_list = G[s, b, w_lo:w_hi]
            base = np.concatenate([[0], np.cumsum(g_list)])
            struct.append(dict(s=s, b=b, w_lo=w_lo, w_hi=w_hi,
                               g_list=g_list, g_base=base,
                               G=int(g_list.sum())))
    # global column offsets
    offG = 0
    off16 = 0
    for sb in struct:
        sb["offG"] = offG
        sb["off16"] = off16
        offG += sb["G"]
        off16 += sb["G"] * 8  # 128 slots / 16
    CG = offG
    Gmax = max((sb["G"] for sb in struct), default=1)

    # per-edge slot assignment (per core)
    idx_all = np.zeros((C, 128, CG * 8), np.int16)
    dloc_all = np.full((C, 128, CG), -1.0, BF16)
    # precompute slot base for each (s,b,w): global slot start
    slot_base = np.zeros((NS, NBLK, W), np.int64)
    for sb in struct:
        s, b = sb["s"], sb["b"]
        for i, w in enumerate(range(sb["w_lo"], sb["w_hi"])):
            slot_base[s, b, w] = (sb["offG"] + sb["g_base"][i]) * 128

    for c in range(C):
        m = t_core == c
        k = keyW[m]
        order = np.argsort(k, kind="stable")
        ks = k[order]
        # rank within each run
        run_start = np.searchsorted(ks, np.arange(nkeys))
        rank = np.arange(len(ks)) - run_start[ks]
        sb_s = ks // (NBLK * W)
        sb_b = (ks // W) % NBLK
        sb_w = ks % W
        slot = slot_base[sb_s, sb_b, sb_w] + rank
        iv = idxv[m][order]
        dv = dloc[m][order]
        # idx wrapped layout: slot j -> (j%16, j//16), replicated x8
        prow = slot % 16
        pcol = slot // 16
        tmp = np.zeros((16, CG * 8), np.int16)
        tmp[prow, pcol] = iv.astype(np.int16)
        idx_all[c] = np.tile(tmp, (8, 1))
        dloc_all[c, slot % 128, slot // 128] = dv.astype(BF16)

    return dict(deg=deg, struct=struct, CG=CG, Gmax=Gmax, NS=NS, W=W,
                bounds=bounds, idx_all=idx_all, dloc_all=dloc_all)


def build_pool_meta(src, dst, batch, part, td_deg, bu_deg, cfg):
    """Layer-2-as-pooled-matmul coefficients.

    out_graph[g] = sum_v M[row(v), g] * hn2[v] + n_g * b2, where hn2 is the
    (AllGathered) dinv*(h1@W2) table in chunked table-row order.  M folds the
    edge aggregation (target-side dinv summed per source node and graph) and
    the self-loop diagonal."""
    C, N, G = cfg["N_CORES"], cfg["N"], cfg["NUM_GRAPHS"]
    gpc = G // C
    R = part["bounds"][-1]
    tr = part["table_row"]
    node_core = part["node_core"]
    batch = np.asarray(batch)

    dinv_td = 1.0 / np.sqrt(td_deg)          # [N] float64
    dinv_bu = 1.0 / np.sqrt(bu_deg)

    M_td = np.zeros((C, R, 128), np.float32)
    M_bu = np.zeros((C, R, 128), np.float32)
    # td: value row src, target dst -> coeff dinv_td[dst] into (core(dst), g(dst))
    c_t = node_core[dst]
    np.add.at(M_td, (c_t, tr[src], batch[dst] - c_t * gpc), dinv_td[dst].astype(np.float32))
    # bu: value row dst, target src -> coeff dinv_bu[src] into (core(src), g(src))
    c_s = node_core[src]
    np.add.at(M_bu, (c_s, tr[dst], batch[src] - c_s * gpc), dinv_bu[src].astype(np.float32))
    # self-loop diagonals: M[row(v), g(v)] += dinv[v] on the core owning v
    allv = np.arange(N)
    c_v = node_core[allv]
    np.add.at(M_td, (c_v, tr[allv], batch[allv] - c_v * gpc), dinv_td[allv].astype(np.float32))
    np.add.at(M_bu, (c_v, tr[allv], batch[allv] - c_v * gpc), dinv_bu[allv].astype(np.float32))
    n_g = np.bincount(batch, minlength=G).astype(np.float32)  # nodes per graph
    return dict(M_td=M_td.astype(BF16), M_bu=M_bu.astype(BF16), n_g=n_g, R=R)


def pool_batches(part, bw=32):
    """Window batches for the pooled-matmul phase, aligned to AG chunks."""
    bounds = part["bounds"]
    batches = []
    for q in range(len(bounds) - 1):
        u0 = bounds[q] // 128
        nwq = (bounds[q + 1] - bounds[q]) // 128
        for s in range(0, nwq, bw):
            batches.append((q, u0 + s, min(bw, nwq - s)))
    return batches


def pack_M(M, batches):
    """Repack [R, G] so each batch's block reads contiguously per partition:
    row p*nw+j holds window (u0+j) partition p."""
    out = np.empty_like(M)
    for (q, u0, nw) in batches:
        blk = M[u0 * 128:(u0 + nw) * 128].reshape(nw, 128, -1)
        out[u0 * 128:(u0 + nw) * 128] = blk.transpose(1, 0, 2).reshape(nw * 128, -1)
    return out


def build_all_inputs(x, edge_index, batch, Ws, bs, cfg):
    """Produce per-core in_maps plus structural metadata."""
    C = cfg["N_CORES"]
    N = cfg["N"]
    src = np.asarray(edge_index[0])
    dst = np.asarray(edge_index[1])
    part = build_partition(batch, cfg,
                           deg_td=np.bincount(dst, minlength=N),
                           deg_bu=np.bincount(src, minlength=N))
    NPC = part["NPC"]
    W = NPC // 128

    td = build_direction_meta(src, dst, part, cfg)   # gather src row, scatter to dst
    bu = build_direction_meta(dst, src, part, cfg)   # reversed
    pool = build_pool_meta(src, dst, batch, part, td["deg"], bu["deg"], cfg)
    batches = pool_batches(part)
    pool["M_td"] = np.stack([pack_M(pool["M_td"][c], batches) for c in range(C)])
    pool["M_bu"] = np.stack([pack_M(pool["M_bu"][c], batches) for c in range(C)])

    Gmax = max(td["Gmax"], bu["Gmax"])
    iota_rep = np.tile(np.arange(128, dtype=np.float32), Gmax)[None, :].repeat(128, 0).astype(BF16)

    # per-core tensors
    in_maps = []
    xT_full = np.ascontiguousarray(np.asarray(x).T)  # [IN, N]
    batch_np = np.asarray(batch)
    ngb2 = np.concatenate([np.outer(pool["n_g"], bs[1]),
                           np.outer(pool["n_g"], bs[3])], axis=1).astype(np.float32)
    for c in range(C):
        lo, hi = part["starts"][c], part["starts"][c + 1]
        cnt = hi - lo
        li = part["node_local"][lo:hi]
        xT = np.zeros((cfg["IN_FEATS"], NPC), BF16)
        xT[:, li] = xT_full[:, lo:hi].astype(BF16)
        deg_t = np.ones((128, W), np.float32)
        deg_b = np.ones((128, W), np.float32)
        deg_t[li % 128, li // 128] = td["deg"][lo:hi].astype(np.float32)
        deg_b[li % 128, li // 128] = bu["deg"][lo:hi].astype(np.float32)
        im = dict(
            xT=xT, ident=np.eye(128, dtype=BF16),
            deg_td=deg_t, deg_bu=deg_b, iota_rep=iota_rep,
            M_td=pool["M_td"][c], M_bu=pool["M_bu"][c],
            ngb2=ngb2[c * part["gpc"]:(c + 1) * part["gpc"]],
            idx_td=td["idx_all"][c], idx_bu=bu["idx_all"][c],
            dstloc_td=td["dloc_all"][c], dstloc_bu=bu["dloc_all"][c],
            W_td1=Ws[0].astype(BF16), W_bu1=Ws[2].astype(BF16),
            W_td2=Ws[1].astype(BF16), W_bu2=Ws[3].astype(BF16),
            b_td1=np.tile(bs[0][None, :], (128, 1)).astype(np.float32),
            b_td2=np.tile(bs[1][None, :], (128, 1)).astype(np.float32),
            b_bu1=np.tile(bs[2][None, :], (128, 1)).astype(np.float32),
            b_bu2=np.tile(bs[3][None, :], (128, 1)).astype(np.float32),
        )
        in_maps.append(im)
    meta = dict(part=part, td=td, bu=bu, Gmax=Gmax, NPC=NPC, W=W, cfg=cfg,
                R=pool["R"], batches=batches)
    return in_maps, meta


# =====================================================================
# Bass program
# =====================================================================

def build_bass(meta):
    import concourse.bacc as bacc
    import concourse.mybir as mybir
    import concourse.tile as tile

    cfg = meta["cfg"]
    C = cfg["N_CORES"]
    NPC, W, Gmax = meta["NPC"], meta["W"], meta["Gmax"]
    IN, HID = cfg["IN_FEATS"], cfg["HIDDEN"]
    NBLK = cfg["NBLK"]
    f32, bf16, i16 = mybir.dt.float32, mybir.dt.bfloat16, mybir.dt.int16

    nc = bacc.Bacc("TRN2", target_bir_lowering=False, debug=False, num_devices=C,
                   num_swdge_queues=4)

    # ---- I/O ----
    ten = {}
    def inp(name, shape, dt):
        ten[name] = nc.dram_tensor(name, shape, dt, kind="ExternalInput")
        return ten[name]

    inp("xT", [IN, NPC], bf16)
    inp("deg_td", [128, W], f32); inp("deg_bu", [128, W], f32)
    inp("iota_rep", [128, Gmax * 128], bf16)
    inp("ident", [128, 128], bf16)
    inp("ngb2", [128, 2 * HID], f32)
    R = meta["R"]
    for d in ("td", "bu"):
        m = meta[d]
        inp(f"idx_{d}", [128, m["CG"] * 8], i16)
        inp(f"dstloc_{d}", [128, m["CG"]], bf16)
        inp(f"M_{d}", [R, 128], bf16)
        inp(f"W_{d}1", [IN, HID], bf16)
        inp(f"W_{d}2", [HID, HID], bf16)
        inp(f"b_{d}1", [128, HID], f32)
        inp(f"b_{d}2", [128, HID], f32)
    out_t = nc.dram_tensor("out", [128, 2 * HID], f32, kind="ExternalOutput")
    dbg = meta.get("dbg")
    if dbg:
        dbg_h1 = {d: nc.dram_tensor(f"dbg_h1_{d}", [NPC, HID], f32, kind="ExternalOutput")
                  for d in ("td", "bu")}
        dbg_m = {d: nc.dram_tensor(f"dbg_m_{d}", [NPC, HID], f32, kind="ExternalOutput")
                 for d in ("td", "bu")}

    # internal DRAM: AG inputs + tables
    ag_in, table = {}, {}
    for d in ("td", "bu"):
        for l in (1, 2):
            ag_in[d, l] = nc.dram_tensor(f"agin_{d}{l}", [NPC, HID], bf16, kind="Internal")
            table[d, l] = nc.dram_tensor(f"table_{d}{l}", [C * NPC, HID], bf16,
                                         kind="Internal", addr_space="Shared")

    rg = [list(range(C))]

    from contextlib import ExitStack
    with tile.TileContext(nc) as tc, ExitStack() as stack:
        def pool(name, bufs, space="SBUF"):
            return stack.enter_context(tc.tile_pool(name=name, bufs=bufs, space=space))

        const = pool("const", 1)
        xt_p = pool("xt", 6)
        hn_p = pool("hn", 4)                 # hn tiles to DRAM
        idx_p = pool("idx", 4)
        dl_p = pool("dl", 4)
        gat_p = pool("gat", 5)               # gathered edge tiles
        oh_p = pool("oh", 3)                 # one-hot tiles
        win_p = pool("win", 5, "PSUM")       # window psum, 4 windows/bank
        epi_p = pool("epi", 6)               # epilogue sbuf tiles
        h1_p = pool("h1", 4)
        t_p = pool("tt", 4)                  # transposes
        mb_p = pool("mb", 2)                 # pooled-matmul M batches
        tb_p = pool("tb", 2)                 # pooled-matmul hn2 batches
        outp = pool("outp", 1)
        hps_p = pool("hps", 2, "PSUM")
        pool_ps = pool("plps", 1, "PSUM")

        # ---- constants in SBUF ----
        iota = const.tile([128, Gmax * 128], bf16, tag="iota")
        nc.sync.dma_start(iota[:], ten["iota_rep"][:])
        Wt = {}
        for d in ("td", "bu"):
            for l, k in ((1, IN), (2, HID)):
                chunks = []
                for kk in range(k // 128):
                    t = const.tile([128, HID], bf16, tag=f"W_{d}{l}_{kk}", name=f"W_{d}{l}_{kk}")
                    nc.sync.dma_start(t[:], ten[f"W_{d}{l}"][kk * 128:(kk + 1) * 128, :])
                    chunks.append(t)
                Wt[d, l] = chunks
        Wcat = []
        for kk in range(IN // 128):
            t = const.tile([128, 2 * HID], bf16, tag=f"Wcat{kk}", name=f"Wcat{kk}")
            nc.sync.dma_start(t[:, 0:HID], ten["W_td1"][kk * 128:(kk + 1) * 128, :])
            nc.sync.dma_start(t[:, HID:2 * HID], ten["W_bu1"][kk * 128:(kk + 1) * 128, :])
            Wcat.append(t)
        bt = {}
        for d in ("td", "bu"):
            for l in (1, 2):
                t = const.tile([128, HID], f32, tag=f"b_{d}{l}", name=f"bt_{d}{l}")
                nc.sync.dma_start(t[:], ten[f"b_{d}{l}"][:])
                bt[d, l] = t
        zrow = const.tile([1, 512], bf16, tag="zrow")
        nc.gpsimd.memset(zrow[:], 0.0)
        ident = const.tile([128, 128], bf16, tag="ident")
        nc.sync.dma_start(ident[:], ten["ident"][:])
        ngb2_t = const.tile([128, 2 * HID], f32, tag="ngb2")
        nc.sync.dma_start(ngb2_t[:], ten["ngb2"][:])

        dinv = {}
        for d in ("td", "bu"):
            degt = const.tile([128, W], f32, tag=f"deg_{d}", name=f"degt_{d}")
            nc.sync.dma_start(degt[:], ten[f"deg_{d}"][:])
            rec = const.tile([128, W], f32, tag=f"rec_{d}", name=f"rec_{d}")
            nc.vector.reciprocal(rec[:], degt[:])
            dv = const.tile([128, W], f32, tag=f"dinv_{d}", name=f"dinv_{d}")
            nc.scalar.activation(dv[:], rec[:], mybir.ActivationFunctionType.Sqrt)
            dinv[d] = dv

        # ---- phase A1: conv1 tables (both directions share xT loads) ----
        cw = meta["part"]["cw"]
        bounds = meta["td"]["bounds"]

        def emit_ag(d, l, q):
            nc.gpsimd.collective_compute(
                "AllGather", mybir.AluOpType.bypass, replica_groups=rg,
                ins=[ag_in[d, l][128 * int(cw[q]):128 * int(cw[q + 1]), :]],
                outs=[table[d, l][bounds[q]:bounds[q + 1], :]])

        nK = IN // 128
        for q0 in range(NBLK):
            for w0 in range(int(cw[q0]), int(cw[q0 + 1]), 4):
                bwn = min(4, int(cw[q0 + 1]) - w0)
                xts = []
                for kk in range(nK):
                    t = xt_p.tile([128, 4 * 128], bf16, tag="xt", name=f"xt_{w0}_{kk}")
                    nc.sync.dma_start(t[:, :bwn * 128],
                                      ten["xT"][kk * 128:(kk + 1) * 128,
                                                w0 * 128:(w0 + bwn) * 128])
                    xts.append(t)
                hnb = {d: hn_p.tile([128, 4, HID], bf16, tag="hnb", name=f"hnb_{d}_{w0}")
                       for d in ("td", "bu")}
                for j in range(bwn):
                    w = w0 + j
                    hps = hps_p.tile([128, 2 * HID], f32, tag="hps")
                    for kk in range(nK):
                        nc.tensor.matmul(hps[:], xts[kk][:, j * 128:(j + 1) * 128],
                                         Wcat[kk][:], start=(kk == 0), stop=(kk == nK - 1))
                    for d, off in (("td", 0), ("bu", HID)):
                        nc.vector.tensor_scalar_mul(hnb[d][:, j, :], hps[:, off:off + HID],
                                                    dinv[d][:, w:w + 1])
                for d in ("td", "bu"):
                    nc.sync.dma_start(
                        ag_in[d, 1][w0 * 128:(w0 + bwn) * 128, :]
                        .rearrange("(j p) f -> p j f", p=128),
                        hnb[d][:, :bwn, :])
            emit_ag("td", 1, q0)
            emit_ag("bu", 1, q0)

        # ---- edge phase for one conv ----
        def edge_phase(d, l):
            m = meta[d]
            first_mm = {}
            last_mm = {}
            # find last (sb_idx, group) per window for stop flags
            for sbi, sb in enumerate(m["struct"]):
                for i, w in enumerate(range(sb["w_lo"], sb["w_hi"])):
                    if sb["g_list"][i] > 0:
                        last_mm[w] = (sbi, int(sb["g_base"][i]) + int(sb["g_list"][i]) - 1)
            quad_tiles = {}
            def win_ap(w):
                q = w // 4
                if q not in quad_tiles:
                    qt = win_p.tile([128, 512], f32, tag="win",
                                    name=f"win_{d}{l}_{q}")
                    nc.tensor.matmul(qt[:], zrow[0:1, 0:128], zrow[0:1, 0:512],
                                     start=True, stop=False, skip_group_check=True)
                    quad_tiles[q] = qt
                return quad_tiles[q][:, (w % 4) * 128:(w % 4 + 1) * 128]
            for sbi, sb in enumerate(m["struct"]):
                G = sb["G"]
                if G == 0:
                    continue
                it = idx_p.tile([128, G * 8], i16, tag="idx")
                nc.sync.dma_start(it[:], ten[f"idx_{d}"][:, sb["off16"]:sb["off16"] + G * 8])
                dlt = dl_p.tile([128, G], bf16, tag="dl")
                nc.sync.dma_start(dlt[:], ten[f"dstloc_{d}"][:, sb["offG"]:sb["offG"] + G])
                gt = gat_p.tile([128, G, 128], bf16, tag="gat")
                blk = table[d, l][m["bounds"][sb["b"]]:m["bounds"][sb["b"] + 1], :]
                qn[0] += 1
                nc.gpsimd.dma_gather(gt[:], blk, it[:], num_idxs=G * 128,
                                     num_idxs_reg=G * 128, elem_size=HID,
                                     single_packet=False, queue_num=qn[0] % 4)
                oh = oh_p.tile([128, G * 128], bf16, tag="oh")
                nc.vector.tensor_tensor(
                    out=oh[:],
                    in0=dlt[:].rearrange("p (g o) -> p g o", o=1).to_broadcast([128, G, 128]),
                    in1=iota[:, :G * 128].rearrange("p (g f) -> p g f", f=128),
                    op=mybir.AluOpType.is_equal)
                for i, w in enumerate(range(sb["w_lo"], sb["w_hi"])):
                    gl = int(sb["g_list"][i])
                    if gl == 0:
                        continue
                    pt = win_ap(w)
                    gb = int(sb["g_base"][i])
                    for g in range(gb, gb + gl):
                        nc.tensor.matmul(
                            pt[:], oh[:, g * 128:(g + 1) * 128], gt[:, g, :],
                            start=False, stop=(last_mm[w] == (sbi, g)),
                            skip_group_check=True)
                # epilogues for completed supers: after last block of super
                if sb["b"] == NBLK - 1:
                    nsw = sb["w_hi"] - sb["w_lo"]
                    hnb = hn_p.tile([128, nsw, HID], bf16, tag="hn_ep")
                    nc.sync.dma_start(
                        hnb[:], ag_in[d, l][sb["w_lo"] * 128:sb["w_hi"] * 128, :]
                        .rearrange("(j p) f -> p j f", p=128))
                    for i, w in enumerate(range(sb["w_lo"], sb["w_hi"])):
                        epilogue(d, l, w, win_ap(w), hnb[:, i, :])
                    quad_tiles.clear()
                    yield sb["w_hi"]
                else:
                    yield None

        def epilogue(d, l, w, pt, hn):
            o1 = epi_p.tile([128, HID], f32, tag="o1")
            nc.vector.scalar_tensor_tensor(
                out=o1[:], in0=pt[:], scalar=dinv[d][:, w:w + 1], in1=bt[d, l][:],
                op0=mybir.AluOpType.mult, op1=mybir.AluOpType.add)
            o2 = epi_p.tile([128, HID], bf16, tag="o2")
            nc.vector.scalar_tensor_tensor(
                out=o2[:], in0=hn, scalar=dinv[d][:, w:w + 1], in1=o1[:],
                op0=mybir.AluOpType.mult, op1=mybir.AluOpType.add)
            if dbg and l == 1:
                mf = epi_p.tile([128, HID], f32, tag="mf")
                nc.vector.tensor_copy(mf[:], pt[:])
                nc.sync.dma_start(dbg_m[d][w * 128:(w + 1) * 128, :], mf[:])
            h1 = h1_p.tile([128, HID], bf16, tag="h1")
            nc.scalar.activation(h1[:], o2[:], mybir.ActivationFunctionType.Relu)
            if dbg:
                h1f = epi_p.tile([128, HID], f32, tag="h1f")
                nc.vector.tensor_copy(h1f[:], h1[:])
                nc.sync.dma_start(dbg_h1[d][w * 128:(w + 1) * 128, :], h1f[:])
            tps = hps_p.tile([128, HID], bf16, tag="hps", name=f"tps_{d}_{w}")
            nc.tensor.transpose(tps[:], h1[:], ident[:])
            h1T = t_p.tile([128, HID], bf16, tag="h1T")
            nc.vector.tensor_copy(h1T[:], tps[:])
            h2 = hps_p.tile([128, HID], f32, tag="hps")
            nc.tensor.matmul(h2[:], h1T[:], Wt[d, 2][0][:], start=True, stop=True)
            hn2 = hn_p.tile([128, HID], bf16, tag="hn2")
            nc.vector.tensor_scalar_mul(hn2[:], h2[:], dinv[d][:, w:w + 1])
            nc.sync.dma_start(ag_in[d, 2][w * 128:(w + 1) * 128, :], hn2[:])

        # ---- layer-2 pooled matmul: out[g] += M_u^T @ hn2_u per table window ----
        batches = meta["batches"]
        last_uq = {}   # d -> (q, u0, nw) of final batch
        for d in ("td", "bu"):
            last_uq[d] = batches[-1]

        def emit_pool(d, q):
            off = 0 if d == "td" else HID
            for (bq, u0, nw) in batches:
                if bq != q:
                    continue
                mt = mb_p.tile([128, nw, HID], bf16, tag="mb")
                nc.scalar.dma_start(
                    mt[:], ten[f"M_{d}"][u0 * 128:(u0 + nw) * 128, :]
                    .rearrange("(p j) g -> p j g", p=128))
                ht = tb_p.tile([128, nw, HID], bf16, tag="tb")
                nc.scalar.dma_start(
                    ht[:], table[d, 2][u0 * 128:(u0 + nw) * 128, :]
                    .rearrange("(j p) f -> p j f", p=128))
                for j in range(nw):
                    is_last = (bq, u0, nw) == last_uq[d] and j == nw - 1
                    nc.tensor.matmul(
                        pool_psum_t[:, off:off + HID],
                        mt[:, j, :], ht[:, j, :],
                        start=False, stop=is_last, skip_group_check=True)

        qn = [0]

        pool_psum_t = pool_ps.tile([128, 2 * HID], f32, tag="pool", name="pool_psum_t")
        nc.tensor.matmul(pool_psum_t[:], zrow[0:1, 0:128], zrow[0:1, 0:2 * HID],
                         start=True, stop=False, skip_group_check=True)

        def run_layer(l):
            gens = {"td": edge_phase("td", l), "bu": edge_phase("bu", l)}
            done = {"td": False, "bu": False}
            next_q = {"td": 0, "bu": 0}
            while not all(done.values()):
                for d in ("td", "bu"):
                    if done[d]:
                        continue
                    try:
                        res = next(gens[d])
                    except StopIteration:
                        done[d] = True
                        res = W
                    if l == 1 and res is not None:
                        while next_q[d] < NBLK and res >= int(cw[next_q[d] + 1]):
                            emit_ag(d, 2, next_q[d])
                            emit_pool(d, next_q[d])
                            next_q[d] += 1

        run_layer(1)

        outsb = outp.tile([128, 2 * HID], f32, tag="out")
        nc.vector.tensor_tensor(out=outsb[:], in0=pool_psum_t[:], in1=ngb2_t[:],
                                op=mybir.AluOpType.add)
        nc.sync.dma_start(out_t[:], outsb[:])

    nc.compile()
    return nc


# =====================================================================
# Entry point
# =====================================================================

def _run(inputs, cfg, trace=False):
    from concourse import bass_utils
    x = np.asarray(inputs["x"], np.float32)
    edge_index = np.asarray(inputs["edge_index"])
    batch = np.asarray(inputs["batch"])
    Ws = [np.asarray(inputs[k], np.float32) for k in ("W_td1", "W_td2", "W_bu1", "W_bu2")]
    bs = [np.asarray(inputs[k], np.float32) for k in ("b_td1", "b_td2", "b_bu1", "b_bu2")]
    in_maps, meta = build_all_inputs(x, edge_index, batch, Ws, bs, cfg)
    nc = build_bass(meta)
    res = bass_utils.run_bass_kernel_spmd(
        nc, in_maps, core_ids=list(range(cfg["N_CORES"])), trace=trace)
    gpc = meta["part"]["gpc"]
    out = np.concatenate([res.results[c]["out"][:gpc] for c in range(cfg["N_CORES"])], axis=0)
    return out.astype(np.float32), res


def kernel(**inputs):
    out, _ = _run(inputs, FULL_CFG, trace=False)
    return out



# revision 25
# speedup vs baseline: 1.6634x; 1.0329x over previous
"""BiGCN (2-layer bidirectional GCN + global add pool) on 8 Trainium2 NeuronCores.

Strategy (hardcoded for the nn_BiGCN_graphcl problem shapes):
  - Nodes are sharded graph-aligned: core c owns graphs [128c, 128c+128) and
    their (contiguous, batch-sorted) node range, padded to a common NPC.
  - Per direction (td / bu), edges are assigned to the core owning their
    target node.  GCNConv is computed as
        out = dinv * (scatter_add(hn[src], dst) + hn) + b,   hn = dinv * (x @ W)
    so no per-edge scaling is needed on device.
  - The hn table ([8*NPC, 128] bf16) is AllGathered between layers; each core
    gathers rows for its edge shard with dma_gather (256B rows), builds a
    staircase one-hot with a DVE is_equal against an iota constant, and
    segment-sums on the TensorEngine into per-window (128-node) PSUM tiles.
  - The SPMD program is identical on all cores: all per-core variation lives
    in uploaded index/data tensors; run lengths are padded to the max across
    cores (pad slots gather row 0 of the block and carry dstloc=-1 so their
    one-hot column is zero).
  - Graph pooling is a second one-hot matmul into a [128 graphs, 128] PSUM
    tile; the host just concatenates the 8 per-core [128, 256] outputs.
"""

import math
import numpy as np
import ml_dtypes

BF16 = ml_dtypes.bfloat16

# ---------------------------------------------------------------- problem cfg
FULL_CFG = dict(
    N=100000, E=1600000, IN_FEATS=256, HIDDEN=128, OUT_FEATS=128,
    NUM_GRAPHS=1024, N_CORES=8, SW=8, NBLK=4,
)


def _round_up(x, m):
    return (x + m - 1) // m * m


# =====================================================================
# Host-side metadata construction
# =====================================================================

def build_partition(batch, cfg, deg_td=None, deg_bu=None):
    """Graph-aligned node partition. Returns dict with per-core node ranges.

    If degree arrays are given, each core's local node order is permuted so
    that per-window (128-node) degree sums cluster just under multiples of
    4*128 edges per (window, src-block) run, minimizing ceil-128 padding."""
    N, C, G = cfg["N"], cfg["N_CORES"], cfg["NUM_GRAPHS"]
    gpc = G // C  # graphs per core
    starts = np.searchsorted(batch, np.arange(0, G + 1, gpc))
    counts = np.diff(starts)
    NPC = max(128, _round_up(int(counts.max()), 128))
    W = NPC // 128
    node_core = np.searchsorted(starts[1:], np.arange(N), side="right")
    node_local = np.arange(N) - starts[node_core]

    if deg_td is not None:
        NBLK = cfg["NBLK"]
        MARGIN = 45 * NBLK  # leave room for cross-core/block-split variance
        for c in range(C):
            lo, hi = starts[c], starts[c + 1]
            cnt = hi - lo
            dt = deg_td[lo:hi].astype(np.int64)
            db = deg_bu[lo:hi].astype(np.int64)
            order = np.argsort(-(dt + db), kind="stable")
            tg_t = np.full(W, dt.sum() / W)
            tg_b = np.full(W, db.sum() / W)
            rem_t = tg_t.astype(np.float64).copy()
            rem_b = tg_b.astype(np.float64).copy()
            room = np.full(W, 128, np.int64)
            assign = np.empty(cnt, np.int64)
            for j in order:
                score = np.minimum(rem_t - dt[j], rem_b - db[j])
                score[room <= 0] = -np.inf
                w = int(np.argmax(score))
                assign[j] = w
                rem_t[w] -= dt[j]
                rem_b[w] -= db[j]
                room[w] -= 1
            # positions: window-major order
            slot_in_w = np.zeros(W, np.int64)
            newloc = np.empty(cnt, np.int64)
            for j in range(cnt):
                w = assign[j]
                newloc[j] = w * 128 + slot_in_w[w]
                slot_in_w[w] += 1
            node_local[lo:hi] = newloc

    # ---- chunk decomposition: 4 window-chunks, sized so per-(window, chunk)
    # gather runs land just under multiples of 128, and each chunk's block of
    # 8*128*w_q table rows stays within int16 index range. ----
    NBLK = cfg["NBLK"]
    mean_w = max(1.0, (deg_td.sum() + deg_bu.sum()) / (2.0 * C * W)) if deg_td is not None else 128.0
    wmax = min(W, (32767 // (128 * C)))

    def padfrac(wb):
        r = wb / W * mean_w  # mean edges per (window, this-chunk) run
        if r <= 0:
            return 0.0
        margin = 1.6 * np.sqrt(r) + 6
        gslots = 128 * np.ceil((r + margin) / 128)
        return (gslots - r) * 1.0

    best = None
    for w1 in range(1, wmax + 1):
        for w2 in range(w1, wmax + 1):
            for w3 in range(w2, wmax + 1):
                w4 = W - w1 - w2 - w3
                if w4 < w3 or w4 > wmax:
                    continue
                cost = padfrac(w1) + padfrac(w2) + padfrac(w3) + padfrac(w4)
                if best is None or cost < best[0]:
                    best = (cost, (w1, w2, w3, w4))
    ws = sorted(best[1], reverse=True) if best else [W]
    # big chunks first: their AG starts earliest, and the LAST chunk (whose
    # pooled-matmul work forms the serial tail) is smallest
    cw = np.concatenate([[0], np.cumsum(ws)])
    assert cw[-1] == W

    chunk_of_w = np.searchsorted(cw[1:], np.arange(W), side="right")
    q = chunk_of_w[np.minimum(node_local // 128, W - 1)]
    rpr = 128 * np.diff(cw)  # rows per rank per chunk
    base = np.concatenate([[0], np.cumsum(rpr * C)])
    table_row = base[q] + node_core * rpr[q] + (node_local - 128 * cw[q])
    bounds = [int(b) for b in base]
    return dict(starts=starts, counts=counts, NPC=NPC, gpc=gpc,
                node_core=node_core.astype(np.int64),
                node_local=node_local.astype(np.int64),
                table_row=table_row.astype(np.int64),
                cw=cw, bounds=bounds)


def build_direction_meta(gather_nodes, target_nodes, part, cfg):
    """Build per-core gather index / dstloc arrays and the uniform group
    structure for one edge direction.

    gather_nodes[e]: node whose table row is gathered for edge e.
    target_nodes[e]: node receiving the contribution.
    """
    N, C = cfg["N"], cfg["N_CORES"]
    SW, NBLK = cfg["SW"], cfg["NBLK"]
    NPC = part["NPC"]
    W = NPC // 128
    NS = (W + SW - 1) // SW
    R = C * NPC

    deg = np.bincount(target_nodes, minlength=N).astype(np.float64) + 1.0

    bounds = part["bounds"]
    assert len(bounds) == NBLK + 1
    assert all(bounds[i + 1] - bounds[i] <= 32767 for i in range(NBLK))
    bounds_arr = np.array(bounds[1:-1])

    tr_g = part["table_row"][gather_nodes]
    t_core = part["node_core"][target_nodes]
    t_local = part["node_local"][target_nodes]
    lw = t_local // 128          # window
    dloc = t_local % 128         # position within window
    blk = np.searchsorted(bounds_arr, tr_g, side="right")
    idxv = tr_g - np.array(bounds[:-1])[blk]
    sup = lw // SW

    # per (core, s, b, w) counts -> uniform G
    keyW = (sup * NBLK + blk) * W + lw  # key within a core
    nkeys = NS * NBLK * W
    counts = np.zeros((C, nkeys), np.int64)
    for c in range(C):
        m = t_core == c
        counts[c] = np.bincount(keyW[m], minlength=nkeys)
    max_counts = counts.max(axis=0).reshape(NS, NBLK, W)

    G = np.ceil(max_counts / 128).astype(np.int64)  # groups per (s,b,w)
    # ensure every window has at least one group (psum must be written)
    for s in range(NS):
        w_lo, w_hi = s * SW, min((s + 1) * SW, W)
        for w in range(w_lo, w_hi):
            if G[s, :, w].sum() == 0:
                G[s, 0, w] = 1
        G[s, :, :w_lo] = 0
        G[s, :, w_hi:] = 0

    # structure: per (s,b): window col bases, totals
    struct = []
    for s in range(NS):
        w_lo, w_hi = s * SW, min((s + 1) * SW, W)
        for b in range(NBLK):
            g_list = G[s, b, w_lo:w_hi]
            base = np.concatenate([[0], np.cumsum(g_list)])
            struct.append(dict(s=s, b=b, w_lo=w_lo, w_hi=w_hi,
                               g_list=g_list, g_base=base,
                               G=int(g_list.sum())))
    # global column offsets
    offG = 0
    off16 = 0
    for sb in struct:
        sb["offG"] = offG
        sb["off16"] = off16
        offG += sb["G"]
        off16 += sb["G"] * 8  # 128 slots / 16
    CG = offG
    Gmax = max((sb["G"] for sb in struct), default=1)

    # per-edge slot assignment (per core)
    idx_all = np.zeros((C, 128, CG * 8), np.int16)
    dloc_all = np.full((C, 128, CG), -1.0, BF16)
    # precompute slot base for each (s,b,w): global slot start
    slot_base = np.zeros((NS, NBLK, W), np.int64)
    for sb in struct:
        s, b = sb["s"], sb["b"]
        for i, w in enumerate(range(sb["w_lo"], sb["w_hi"])):
            slot_base[s, b, w] = (sb["offG"] + sb["g_base"][i]) * 128

    for c in range(C):
        m = t_core == c
        k = keyW[m]
        order = np.argsort(k, kind="stable")
        ks = k[order]
        # rank within each run
        run_start = np.searchsorted(ks, np.arange(nkeys))
        rank = np.arange(len(ks)) - run_start[ks]
        sb_s = ks // (NBLK * W)
        sb_b = (ks // W) % NBLK
        sb_w = ks % W
        slot = slot_base[sb_s, sb_b, sb_w] + rank
        iv = idxv[m][order]
        dv = dloc[m][order]
        # idx wrapped layout: slot j -> (j%16, j//16), replicated x8
        prow = slot % 16
        pcol = slot // 16
        tmp = np.zeros((16, CG * 8), np.int16)
        tmp[prow, pcol] = iv.astype(np.int16)
        idx_all[c] = np.tile(tmp, (8, 1))
        dloc_all[c, slot % 128, slot // 128] = dv.astype(BF16)

    return dict(deg=deg, struct=struct, CG=CG, Gmax=Gmax, NS=NS, W=W,
                bounds=bounds, idx_all=idx_all, dloc_all=dloc_all)


def build_pool_meta(src, dst, batch, part, td_deg, bu_deg, cfg):
    """Layer-2-as-pooled-matmul coefficients.

    out_graph[g] = sum_v M[row(v), g] * hn2[v] + n_g * b2, where hn2 is the
    (AllGathered) dinv*(h1@W2) table in chunked table-row order.  M folds the
    edge aggregation (target-side dinv summed per source node and graph) and
    the self-loop diagonal."""
    C, N, G = cfg["N_CORES"], cfg["N"], cfg["NUM_GRAPHS"]
    gpc = G // C
    R = part["bounds"][-1]
    tr = part["table_row"]
    node_core = part["node_core"]
    batch = np.asarray(batch)

    dinv_td = 1.0 / np.sqrt(td_deg)          # [N] float64
    dinv_bu = 1.0 / np.sqrt(bu_deg)

    M_td = np.zeros((C, R, 128), np.float32)
    M_bu = np.zeros((C, R, 128), np.float32)
    # td: value row src, target dst -> coeff dinv_td[dst] into (core(dst), g(dst))
    c_t = node_core[dst]
    np.add.at(M_td, (c_t, tr[src], batch[dst] - c_t * gpc), dinv_td[dst].astype(np.float32))
    # bu: value row dst, target src -> coeff dinv_bu[src] into (core(src), g(src))
    c_s = node_core[src]
    np.add.at(M_bu, (c_s, tr[dst], batch[src] - c_s * gpc), dinv_bu[src].astype(np.float32))
    # self-loop diagonals: M[row(v), g(v)] += dinv[v] on the core owning v
    allv = np.arange(N)
    c_v = node_core[allv]
    np.add.at(M_td, (c_v, tr[allv], batch[allv] - c_v * gpc), dinv_td[allv].astype(np.float32))
    np.add.at(M_bu, (c_v, tr[allv], batch[allv] - c_v * gpc), dinv_bu[allv].astype(np.float32))
    n_g = np.bincount(batch, minlength=G).astype(np.float32)  # nodes per graph
    return dict(M_td=M_td.astype(BF16), M_bu=M_bu.astype(BF16), n_g=n_g, R=R)


def pool_batches(part, bw=32):
    """Window batches for the pooled-matmul phase, aligned to AG chunks."""
    bounds = part["bounds"]
    batches = []
    for q in range(len(bounds) - 1):
        u0 = bounds[q] // 128
        nwq = (bounds[q + 1] - bounds[q]) // 128
        for s in range(0, nwq, bw):
            batches.append((q, u0 + s, min(bw, nwq - s)))
    return batches


def pack_M(M, batches):
    """Repack [R, G] so each batch's block reads contiguously per partition:
    row p*nw+j holds window (u0+j) partition p."""
    out = np.empty_like(M)
    for (q, u0, nw) in batches:
        blk = M[u0 * 128:(u0 + nw) * 128].reshape(nw, 128, -1)
        out[u0 * 128:(u0 + nw) * 128] = blk.transpose(1, 0, 2).reshape(nw * 128, -1)
    return out


def build_all_inputs(x, edge_index, batch, Ws, bs, cfg):
    """Produce per-core in_maps plus structural metadata."""
    C = cfg["N_CORES"]
    N = cfg["N"]
    src = np.asarray(edge_index[0])
    dst = np.asarray(edge_index[1])
    part = build_partition(batch, cfg,
                           deg_td=np.bincount(dst, minlength=N),
                           deg_bu=np.bincount(src, minlength=N))
    NPC = part["NPC"]
    W = NPC // 128

    td = build_direction_meta(src, dst, part, cfg)   # gather src row, scatter to dst
    bu = build_direction_meta(dst, src, part, cfg)   # reversed
    pool = build_pool_meta(src, dst, batch, part, td["deg"], bu["deg"], cfg)
    batches = pool_batches(part, bw=16)
    pool["M_td"] = np.stack([pack_M(pool["M_td"][c], batches) for c in range(C)])
    pool["M_bu"] = np.stack([pack_M(pool["M_bu"][c], batches) for c in range(C)])

    Gmax = max(td["Gmax"], bu["Gmax"])
    iota_rep = np.tile(np.arange(128, dtype=np.float32), Gmax)[None, :].repeat(128, 0).astype(BF16)

    # per-core tensors
    in_maps = []
    xT_full = np.ascontiguousarray(np.asarray(x).T)  # [IN, N]
    batch_np = np.asarray(batch)
    ngb2 = np.concatenate([np.outer(pool["n_g"], bs[1]),
                           np.outer(pool["n_g"], bs[3])], axis=1).astype(np.float32)
    for c in range(C):
        lo, hi = part["starts"][c], part["starts"][c + 1]
        cnt = hi - lo
        li = part["node_local"][lo:hi]
        xT = np.zeros((cfg["IN_FEATS"], NPC), BF16)
        xT[:, li] = xT_full[:, lo:hi].astype(BF16)
        deg_t = np.ones((128, W), np.float32)
        deg_b = np.ones((128, W), np.float32)
        deg_t[li % 128, li // 128] = td["deg"][lo:hi].astype(np.float32)
        deg_b[li % 128, li // 128] = bu["deg"][lo:hi].astype(np.float32)
        im = dict(
            xT=xT, ident=np.eye(128, dtype=BF16),
            deg_td=deg_t, deg_bu=deg_b, iota_rep=iota_rep,
            M_td=pool["M_td"][c], M_bu=pool["M_bu"][c],
            ngb2=ngb2[c * part["gpc"]:(c + 1) * part["gpc"]],
            idx_td=td["idx_all"][c], idx_bu=bu["idx_all"][c],
            dstloc_td=td["dloc_all"][c], dstloc_bu=bu["dloc_all"][c],
            W_td1=Ws[0].astype(BF16), W_bu1=Ws[2].astype(BF16),
            W_td2=Ws[1].astype(BF16), W_bu2=Ws[3].astype(BF16),
            b_td1=np.tile(bs[0][None, :], (128, 1)).astype(np.float32),
            b_td2=np.tile(bs[1][None, :], (128, 1)).astype(np.float32),
            b_bu1=np.tile(bs[2][None, :], (128, 1)).astype(np.float32),
            b_bu2=np.tile(bs[3][None, :], (128, 1)).astype(np.float32),
        )
        in_maps.append(im)
    meta = dict(part=part, td=td, bu=bu, Gmax=Gmax, NPC=NPC, W=W, cfg=cfg,
                R=pool["R"], batches=batches)
    return in_maps, meta


# =====================================================================
# Bass program
# =====================================================================

def build_bass(meta):
    import concourse.bacc as bacc
    import concourse.mybir as mybir
    import concourse.tile as tile

    cfg = meta["cfg"]
    C = cfg["N_CORES"]
    NPC, W, Gmax = meta["NPC"], meta["W"], meta["Gmax"]
    IN, HID = cfg["IN_FEATS"], cfg["HIDDEN"]
    NBLK = cfg["NBLK"]
    f32, bf16, i16 = mybir.dt.float32, mybir.dt.bfloat16, mybir.dt.int16

    nc = bacc.Bacc("TRN2", target_bir_lowering=False, debug=False, num_devices=C,
                   num_swdge_queues=4)

    # ---- I/O ----
    ten = {}
    def inp(name, shape, dt):
        ten[name] = nc.dram_tensor(name, shape, dt, kind="ExternalInput")
        return ten[name]

    inp("xT", [IN, NPC], bf16)
    inp("deg_td", [128, W], f32); inp("deg_bu", [128, W], f32)
    inp("iota_rep", [128, Gmax * 128], bf16)
    inp("ident", [128, 128], bf16)
    inp("ngb2", [128, 2 * HID], f32)
    R = meta["R"]
    for d in ("td", "bu"):
        m = meta[d]
        inp(f"idx_{d}", [128, m["CG"] * 8], i16)
        inp(f"dstloc_{d}", [128, m["CG"]], bf16)
        inp(f"M_{d}", [R, 128], bf16)
        inp(f"W_{d}1", [IN, HID], bf16)
        inp(f"W_{d}2", [HID, HID], bf16)
        inp(f"b_{d}1", [128, HID], f32)
        inp(f"b_{d}2", [128, HID], f32)
    out_t = nc.dram_tensor("out", [128, 2 * HID], f32, kind="ExternalOutput")
    dbg = meta.get("dbg")
    if dbg:
        dbg_h1 = {d: nc.dram_tensor(f"dbg_h1_{d}", [NPC, HID], f32, kind="ExternalOutput")
                  for d in ("td", "bu")}
        dbg_m = {d: nc.dram_tensor(f"dbg_m_{d}", [NPC, HID], f32, kind="ExternalOutput")
                 for d in ("td", "bu")}

    # internal DRAM: AG inputs + tables
    ag_in, table = {}, {}
    for d in ("td", "bu"):
        for l in (1, 2):
            ag_in[d, l] = nc.dram_tensor(f"agin_{d}{l}", [NPC, HID], bf16, kind="Internal")
            table[d, l] = nc.dram_tensor(f"table_{d}{l}", [C * NPC, HID], bf16,
                                         kind="Internal", addr_space="Shared")

    rg = [list(range(C))]

    from contextlib import ExitStack
    with tile.TileContext(nc) as tc, ExitStack() as stack:
        def pool(name, bufs, space="SBUF"):
            return stack.enter_context(tc.tile_pool(name=name, bufs=bufs, space=space))

        const = pool("const", 1)
        xt_p = pool("xt", 6)
        hn_p = pool("hn", 4)                 # hn tiles to DRAM
        idx_p = pool("idx", 8)
        dl_p = pool("dl", 8)
        gat_p = pool("gat", 8)               # gathered edge tiles
        oh_p = pool("oh", 3)                 # one-hot tiles
        win_p = pool("win", 6, "PSUM")       # window psum, 4 windows/bank
        epi_p = pool("epi", 6)               # epilogue sbuf tiles
        h1_p = pool("h1", 4)
        t_p = pool("tt", 4)                  # transposes
        mb_p = pool("mb", 2)                 # pooled-matmul M batches
        tb_p = pool("tb", 2)                 # pooled-matmul hn2 batches
        outp = pool("outp", 1)
        hps_p = pool("hps", 1, "PSUM")
        pool_ps = pool("plps", 1, "PSUM")

        # ---- constants in SBUF ----
        iota = const.tile([128, Gmax * 128], bf16, tag="iota")
        nc.sync.dma_start(iota[:], ten["iota_rep"][:])
        Wt = {}
        for d in ("td", "bu"):
            for l, k in ((1, IN), (2, HID)):
                chunks = []
                for kk in range(k // 128):
                    t = const.tile([128, HID], bf16, tag=f"W_{d}{l}_{kk}", name=f"W_{d}{l}_{kk}")
                    nc.sync.dma_start(t[:], ten[f"W_{d}{l}"][kk * 128:(kk + 1) * 128, :])
                    chunks.append(t)
                Wt[d, l] = chunks
        Wcat = []
        for kk in range(IN // 128):
            t = const.tile([128, 2 * HID], bf16, tag=f"Wcat{kk}", name=f"Wcat{kk}")
            nc.sync.dma_start(t[:, 0:HID], ten["W_td1"][kk * 128:(kk + 1) * 128, :])
            nc.sync.dma_start(t[:, HID:2 * HID], ten["W_bu1"][kk * 128:(kk + 1) * 128, :])
            Wcat.append(t)
        bt = {}
        for d in ("td", "bu"):
            for l in (1, 2):
                t = const.tile([128, HID], f32, tag=f"b_{d}{l}", name=f"bt_{d}{l}")
                nc.sync.dma_start(t[:], ten[f"b_{d}{l}"][:])
                bt[d, l] = t
        zrow = const.tile([1, 512], bf16, tag="zrow")
        nc.gpsimd.memset(zrow[:], 0.0)
        ident = const.tile([128, 128], bf16, tag="ident")
        nc.sync.dma_start(ident[:], ten["ident"][:])
        ngb2_t = const.tile([128, 2 * HID], f32, tag="ngb2")
        nc.sync.dma_start(ngb2_t[:], ten["ngb2"][:])

        dinv = {}
        for d in ("td", "bu"):
            degt = const.tile([128, W], f32, tag=f"deg_{d}", name=f"degt_{d}")
            nc.sync.dma_start(degt[:], ten[f"deg_{d}"][:])
            rec = const.tile([128, W], f32, tag=f"rec_{d}", name=f"rec_{d}")
            nc.vector.reciprocal(rec[:], degt[:])
            dv = const.tile([128, W], f32, tag=f"dinv_{d}", name=f"dinv_{d}")
            nc.scalar.activation(dv[:], rec[:], mybir.ActivationFunctionType.Sqrt)
            dinv[d] = dv

        # ---- phase A1: conv1 tables (both directions share xT loads) ----
        cw = meta["part"]["cw"]
        bounds = meta["td"]["bounds"]

        def emit_ag(d, l, q):
            nc.gpsimd.collective_compute(
                "AllGather", mybir.AluOpType.bypass, replica_groups=rg,
                ins=[ag_in[d, l][128 * int(cw[q]):128 * int(cw[q + 1]), :]],
                outs=[table[d, l][bounds[q]:bounds[q + 1], :]])

        nK = IN // 128
        for q0 in range(NBLK):
            for w0 in range(int(cw[q0]), int(cw[q0 + 1]), 4):
                bwn = min(4, int(cw[q0 + 1]) - w0)
                xts = []
                for kk in range(nK):
                    t = xt_p.tile([128, 4 * 128], bf16, tag="xt", name=f"xt_{w0}_{kk}")
                    nc.sync.dma_start(t[:, :bwn * 128],
                                      ten["xT"][kk * 128:(kk + 1) * 128,
                                                w0 * 128:(w0 + bwn) * 128])
                    xts.append(t)
                hnb = {d: hn_p.tile([128, 4, HID], bf16, tag="hnb", name=f"hnb_{d}_{w0}")
                       for d in ("td", "bu")}
                for j in range(bwn):
                    w = w0 + j
                    hps = hps_p.tile([128, 2 * HID], f32, tag="hps")
                    for kk in range(nK):
                        nc.tensor.matmul(hps[:], xts[kk][:, j * 128:(j + 1) * 128],
                                         Wcat[kk][:], start=(kk == 0), stop=(kk == nK - 1))
                    for d, off in (("td", 0), ("bu", HID)):
                        nc.vector.tensor_scalar_mul(hnb[d][:, j, :], hps[:, off:off + HID],
                                                    dinv[d][:, w:w + 1])
                for d in ("td", "bu"):
                    nc.sync.dma_start(
                        ag_in[d, 1][w0 * 128:(w0 + bwn) * 128, :]
                        .rearrange("(j p) f -> p j f", p=128),
                        hnb[d][:, :bwn, :])
            emit_ag("td", 1, q0)
            emit_ag("bu", 1, q0)

        # ---- edge phase for one conv ----
        def edge_phase(d, l):
            m = meta[d]
            first_mm = {}
            last_mm = {}
            # find last (sb_idx, group) per window for stop flags
            for sbi, sb in enumerate(m["struct"]):
                for i, w in enumerate(range(sb["w_lo"], sb["w_hi"])):
                    if sb["g_list"][i] > 0:
                        last_mm[w] = (sbi, int(sb["g_base"][i]) + int(sb["g_list"][i]) - 1)
            quad_tiles = {}
            def win_ap(w):
                q = w // 4
                if q not in quad_tiles:
                    qt = win_p.tile([128, 512], f32, tag="win",
                                    name=f"win_{d}{l}_{q}")
                    nc.tensor.matmul(qt[:], zrow[0:1, 0:128], zrow[0:1, 0:512],
                                     start=True, stop=False, skip_group_check=True)
                    quad_tiles[q] = qt
                return quad_tiles[q][:, (w % 4) * 128:(w % 4 + 1) * 128]
            for sbi, sb in enumerate(m["struct"]):
                G = sb["G"]
                if G == 0:
                    continue
                it = idx_p.tile([128, G * 8], i16, tag="idx")
                nc.sync.dma_start(it[:], ten[f"idx_{d}"][:, sb["off16"]:sb["off16"] + G * 8])
                dlt = dl_p.tile([128, G], bf16, tag="dl")
                nc.sync.dma_start(dlt[:], ten[f"dstloc_{d}"][:, sb["offG"]:sb["offG"] + G])
                gt = gat_p.tile([128, G, 128], bf16, tag="gat")
                blk = table[d, l][m["bounds"][sb["b"]]:m["bounds"][sb["b"] + 1], :]
                qn[0] += 1
                nc.gpsimd.dma_gather(gt[:], blk, it[:], num_idxs=G * 128,
                                     num_idxs_reg=G * 128, elem_size=HID,
                                     single_packet=False, queue_num=qn[0] % 4)
                oh = oh_p.tile([128, G * 128], bf16, tag="oh")
                nc.vector.tensor_tensor(
                    out=oh[:],
                    in0=dlt[:].rearrange("p (g o) -> p g o", o=1).to_broadcast([128, G, 128]),
                    in1=iota[:, :G * 128].rearrange("p (g f) -> p g f", f=128),
                    op=mybir.AluOpType.is_equal)
                for i, w in enumerate(range(sb["w_lo"], sb["w_hi"])):
                    gl = int(sb["g_list"][i])
                    if gl == 0:
                        continue
                    pt = win_ap(w)
                    gb = int(sb["g_base"][i])
                    for g in range(gb, gb + gl):
                        nc.tensor.matmul(
                            pt[:], oh[:, g * 128:(g + 1) * 128], gt[:, g, :],
                            start=False, stop=(last_mm[w] == (sbi, g)),
                            skip_group_check=True)
                # epilogues for completed supers: after last block of super
                if sb["b"] == NBLK - 1:
                    nsw = sb["w_hi"] - sb["w_lo"]
                    hnb = hn_p.tile([128, nsw, HID], bf16, tag="hn_ep")
                    nc.sync.dma_start(
                        hnb[:], ag_in[d, l][sb["w_lo"] * 128:sb["w_hi"] * 128, :]
                        .rearrange("(j p) f -> p j f", p=128))
                    for i, w in enumerate(range(sb["w_lo"], sb["w_hi"])):
                        epilogue(d, l, w, win_ap(w), hnb[:, i, :])
                    quad_tiles.clear()
                    yield sb["w_hi"]
                else:
                    yield None

        def epilogue(d, l, w, pt, hn):
            o1 = epi_p.tile([128, HID], f32, tag="o1")
            nc.vector.scalar_tensor_tensor(
                out=o1[:], in0=pt[:], scalar=dinv[d][:, w:w + 1], in1=bt[d, l][:],
                op0=mybir.AluOpType.mult, op1=mybir.AluOpType.add)
            o2 = epi_p.tile([128, HID], bf16, tag="o2")
            nc.vector.scalar_tensor_tensor(
                out=o2[:], in0=hn, scalar=dinv[d][:, w:w + 1], in1=o1[:],
                op0=mybir.AluOpType.mult, op1=mybir.AluOpType.add)
            if dbg and l == 1:
                mf = epi_p.tile([128, HID], f32, tag="mf")
                nc.vector.tensor_copy(mf[:], pt[:])
                nc.sync.dma_start(dbg_m[d][w * 128:(w + 1) * 128, :], mf[:])
            h1 = h1_p.tile([128, HID], bf16, tag="h1")
            nc.scalar.activation(h1[:], o2[:], mybir.ActivationFunctionType.Relu)
            if dbg:
                h1f = epi_p.tile([128, HID], f32, tag="h1f")
                nc.vector.tensor_copy(h1f[:], h1[:])
                nc.sync.dma_start(dbg_h1[d][w * 128:(w + 1) * 128, :], h1f[:])
            tps = hps_p.tile([128, HID], bf16, tag="hps", name=f"tps_{d}_{w}")
            nc.tensor.transpose(tps[:], h1[:], ident[:])
            h1T = t_p.tile([128, HID], bf16, tag="h1T")
            nc.vector.tensor_copy(h1T[:], tps[:])
            h2 = hps_p.tile([128, HID], f32, tag="hps")
            nc.tensor.matmul(h2[:], h1T[:], Wt[d, 2][0][:], start=True, stop=True)
            hn2 = hn_p.tile([128, HID], bf16, tag="hn2")
            nc.vector.tensor_scalar_mul(hn2[:], h2[:], dinv[d][:, w:w + 1])
            nc.sync.dma_start(ag_in[d, 2][w * 128:(w + 1) * 128, :], hn2[:])

        # ---- layer-2 pooled matmul: out[g] += M_u^T @ hn2_u per table window ----
        batches = meta["batches"]
        last_uq = {}   # d -> (q, u0, nw) of final batch
        for d in ("td", "bu"):
            last_uq[d] = batches[-1]

        def emit_pool(d, q):
            off = 0 if d == "td" else HID
            for (bq, u0, nw) in batches:
                if bq != q:
                    continue
                mt = mb_p.tile([128, nw, HID], bf16, tag="mb")
                nc.scalar.dma_start(
                    mt[:], ten[f"M_{d}"][u0 * 128:(u0 + nw) * 128, :]
                    .rearrange("(p j) g -> p j g", p=128))
                ht = tb_p.tile([128, nw, HID], bf16, tag="tb")
                nc.scalar.dma_start(
                    ht[:], table[d, 2][u0 * 128:(u0 + nw) * 128, :]
                    .rearrange("(j p) f -> p j f", p=128))
                for j in range(nw):
                    is_last = (bq, u0, nw) == last_uq[d] and j == nw - 1
                    nc.tensor.matmul(
                        pool_psum_t[:, off:off + HID],
                        mt[:, j, :], ht[:, j, :],
                        start=False, stop=is_last, skip_group_check=True)

        qn = [0]

        pool_psum_t = pool_ps.tile([128, 2 * HID], f32, tag="pool", name="pool_psum_t")
        nc.tensor.matmul(pool_psum_t[:], zrow[0:1, 0:128], zrow[0:1, 0:2 * HID],
                         start=True, stop=False, skip_group_check=True)

        def run_layer(l):
            gens = {"td": edge_phase("td", l), "bu": edge_phase("bu", l)}
            done = {"td": False, "bu": False}
            next_q = {"td": 0, "bu": 0}
            while not all(done.values()):
                for d in ("td", "bu"):
                    if done[d]:
                        continue
                    try:
                        res = next(gens[d])
                    except StopIteration:
                        done[d] = True
                        res = W
                    if l == 1 and res is not None:
                        while next_q[d] < NBLK and res >= int(cw[next_q[d] + 1]):
                            emit_ag(d, 2, next_q[d])
                            emit_pool(d, next_q[d])
                            next_q[d] += 1

        run_layer(1)

        outsb = outp.tile([128, 2 * HID], f32, tag="out")
        nc.vector.tensor_tensor(out=outsb[:], in0=pool_psum_t[:], in1=ngb2_t[:],
                                op=mybir.AluOpType.add)
        nc.sync.dma_start(out_t[:], outsb[:])

    nc.compile()
    return nc


# =====================================================================
# Entry point
# =====================================================================

def _run(inputs, cfg, trace=False):
    from concourse import bass_utils
    x = np.asarray(inputs["x"], np.float32)
    edge_index = np.asarray(inputs["edge_index"])
    batch = np.asarray(inputs["batch"])
    Ws = [np.asarray(inputs[k], np.float32) for k in ("W_td1", "W_td2", "W_bu1", "W_bu2")]
    bs = [np.asarray(inputs[k], np.float32) for k in ("b_td1", "b_td2", "b_bu1", "b_bu2")]
    in_maps, meta = build_all_inputs(x, edge_index, batch, Ws, bs, cfg)
    nc = build_bass(meta)
    res = bass_utils.run_bass_kernel_spmd(
        nc, in_maps, core_ids=list(range(cfg["N_CORES"])), trace=trace)
    gpc = meta["part"]["gpc"]
    out = np.concatenate([res.results[c]["out"][:gpc] for c in range(cfg["N_CORES"])], axis=0)
    return out.astype(np.float32), res


def kernel(**inputs):
    out, _ = _run(inputs, FULL_CFG, trace=False)
    return out



# revision 26
# speedup vs baseline: 1.6900x; 1.0160x over previous
"""BiGCN (2-layer bidirectional GCN + global add pool) on 8 Trainium2 NeuronCores.

Strategy (hardcoded for the nn_BiGCN_graphcl problem shapes):
  - Nodes are sharded graph-aligned: core c owns graphs [128c, 128c+128) and
    their (contiguous, batch-sorted) node range, padded to a common NPC.
  - Per direction (td / bu), edges are assigned to the core owning their
    target node.  GCNConv is computed as
        out = dinv * (scatter_add(hn[src], dst) + hn) + b,   hn = dinv * (x @ W)
    so no per-edge scaling is needed on device.
  - The hn table ([8*NPC, 128] bf16) is AllGathered between layers; each core
    gathers rows for its edge shard with dma_gather (256B rows), builds a
    staircase one-hot with a DVE is_equal against an iota constant, and
    segment-sums on the TensorEngine into per-window (128-node) PSUM tiles.
  - The SPMD program is identical on all cores: all per-core variation lives
    in uploaded index/data tensors; run lengths are padded to the max across
    cores (pad slots gather row 0 of the block and carry dstloc=-1 so their
    one-hot column is zero).
  - Graph pooling is a second one-hot matmul into a [128 graphs, 128] PSUM
    tile; the host just concatenates the 8 per-core [128, 256] outputs.
"""

import math
import numpy as np
import ml_dtypes

BF16 = ml_dtypes.bfloat16

# ---------------------------------------------------------------- problem cfg
FULL_CFG = dict(
    N=100000, E=1600000, IN_FEATS=256, HIDDEN=128, OUT_FEATS=128,
    NUM_GRAPHS=1024, N_CORES=8, SW=8, NBLK=4,
)


def _round_up(x, m):
    return (x + m - 1) // m * m


# =====================================================================
# Host-side metadata construction
# =====================================================================

def build_partition(batch, cfg, deg_td=None, deg_bu=None):
    """Graph-aligned node partition. Returns dict with per-core node ranges.

    If degree arrays are given, each core's local node order is permuted so
    that per-window (128-node) degree sums cluster just under multiples of
    4*128 edges per (window, src-block) run, minimizing ceil-128 padding."""
    N, C, G = cfg["N"], cfg["N_CORES"], cfg["NUM_GRAPHS"]
    gpc = G // C  # graphs per core
    starts = np.searchsorted(batch, np.arange(0, G + 1, gpc))
    counts = np.diff(starts)
    NPC = max(128, _round_up(int(counts.max()), 128))
    W = NPC // 128
    node_core = np.searchsorted(starts[1:], np.arange(N), side="right")
    node_local = np.arange(N) - starts[node_core]

    if deg_td is not None:
        NBLK = cfg["NBLK"]
        MARGIN = 45 * NBLK  # leave room for cross-core/block-split variance
        for c in range(C):
            lo, hi = starts[c], starts[c + 1]
            cnt = hi - lo
            dt = deg_td[lo:hi].astype(np.int64)
            db = deg_bu[lo:hi].astype(np.int64)
            order = np.argsort(-(dt + db), kind="stable")
            tg_t = np.full(W, dt.sum() / W)
            tg_b = np.full(W, db.sum() / W)
            rem_t = tg_t.astype(np.float64).copy()
            rem_b = tg_b.astype(np.float64).copy()
            room = np.full(W, 128, np.int64)
            assign = np.empty(cnt, np.int64)
            for j in order:
                score = np.minimum(rem_t - dt[j], rem_b - db[j])
                score[room <= 0] = -np.inf
                w = int(np.argmax(score))
                assign[j] = w
                rem_t[w] -= dt[j]
                rem_b[w] -= db[j]
                room[w] -= 1
            # positions: window-major order
            slot_in_w = np.zeros(W, np.int64)
            newloc = np.empty(cnt, np.int64)
            for j in range(cnt):
                w = assign[j]
                newloc[j] = w * 128 + slot_in_w[w]
                slot_in_w[w] += 1
            node_local[lo:hi] = newloc

    # ---- chunk decomposition: 4 window-chunks, sized so per-(window, chunk)
    # gather runs land just under multiples of 128, and each chunk's block of
    # 8*128*w_q table rows stays within int16 index range. ----
    NBLK = cfg["NBLK"]
    mean_w = max(1.0, (deg_td.sum() + deg_bu.sum()) / (2.0 * C * W)) if deg_td is not None else 128.0
    wmax = min(W, (32767 // (128 * C)))

    def padfrac(wb):
        r = wb / W * mean_w  # mean edges per (window, this-chunk) run
        if r <= 0:
            return 0.0
        margin = 1.6 * np.sqrt(r) + 6
        gslots = 128 * np.ceil((r + margin) / 128)
        return (gslots - r) * 1.0

    best = None
    for w1 in range(1, wmax + 1):
        for w2 in range(w1, wmax + 1):
            for w3 in range(w2, wmax + 1):
                w4 = W - w1 - w2 - w3
                if w4 < w3 or w4 > wmax:
                    continue
                cost = padfrac(w1) + padfrac(w2) + padfrac(w3) + padfrac(w4)
                if best is None or cost < best[0]:
                    best = (cost, (w1, w2, w3, w4))
    ws = sorted(best[1], reverse=True) if best else [W]
    # big chunks first: their AG starts earliest, and the LAST chunk (whose
    # pooled-matmul work forms the serial tail) is smallest
    cw = np.concatenate([[0], np.cumsum(ws)])
    assert cw[-1] == W

    chunk_of_w = np.searchsorted(cw[1:], np.arange(W), side="right")
    q = chunk_of_w[np.minimum(node_local // 128, W - 1)]
    rpr = 128 * np.diff(cw)  # rows per rank per chunk
    base = np.concatenate([[0], np.cumsum(rpr * C)])
    table_row = base[q] + node_core * rpr[q] + (node_local - 128 * cw[q])
    bounds = [int(b) for b in base]
    return dict(starts=starts, counts=counts, NPC=NPC, gpc=gpc,
                node_core=node_core.astype(np.int64),
                node_local=node_local.astype(np.int64),
                table_row=table_row.astype(np.int64),
                cw=cw, bounds=bounds)


def build_direction_meta(gather_nodes, target_nodes, part, cfg):
    """Build per-core gather index / dstloc arrays and the uniform group
    structure for one edge direction.

    gather_nodes[e]: node whose table row is gathered for edge e.
    target_nodes[e]: node receiving the contribution.
    """
    N, C = cfg["N"], cfg["N_CORES"]
    SW, NBLK = cfg["SW"], cfg["NBLK"]
    NPC = part["NPC"]
    W = NPC // 128
    NS = (W + SW - 1) // SW
    R = C * NPC

    deg = np.bincount(target_nodes, minlength=N).astype(np.float64) + 1.0

    bounds = part["bounds"]
    assert len(bounds) == NBLK + 1
    assert all(bounds[i + 1] - bounds[i] <= 32767 for i in range(NBLK))
    bounds_arr = np.array(bounds[1:-1])

    tr_g = part["table_row"][gather_nodes]
    t_core = part["node_core"][target_nodes]
    t_local = part["node_local"][target_nodes]
    lw = t_local // 128          # window
    dloc = t_local % 128         # position within window
    blk = np.searchsorted(bounds_arr, tr_g, side="right")
    idxv = tr_g - np.array(bounds[:-1])[blk]
    sup = lw // SW

    # per (core, s, b, w) counts -> uniform G
    keyW = (sup * NBLK + blk) * W + lw  # key within a core
    nkeys = NS * NBLK * W
    counts = np.zeros((C, nkeys), np.int64)
    for c in range(C):
        m = t_core == c
        counts[c] = np.bincount(keyW[m], minlength=nkeys)
    max_counts = counts.max(axis=0).reshape(NS, NBLK, W)

    G = np.ceil(max_counts / 128).astype(np.int64)  # groups per (s,b,w)
    # ensure every window has at least one group (psum must be written)
    for s in range(NS):
        w_lo, w_hi = s * SW, min((s + 1) * SW, W)
        for w in range(w_lo, w_hi):
            if G[s, :, w].sum() == 0:
                G[s, 0, w] = 1
        G[s, :, :w_lo] = 0
        G[s, :, w_hi:] = 0

    # structure: per (s,b): window col bases, totals
    struct = []
    for s in range(NS):
        w_lo, w_hi = s * SW, min((s + 1) * SW, W)
        for b in range(NBLK):
            g_list = G[s, b, w_lo:w_hi]
            base = np.concatenate([[0], np.cumsum(g_list)])
            struct.append(dict(s=s, b=b, w_lo=w_lo, w_hi=w_hi,
                               g_list=g_list, g_base=base,
                               G=int(g_list.sum())))
    # global column offsets
    offG = 0
    off16 = 0
    for sb in struct:
        sb["offG"] = offG
        sb["off16"] = off16
        offG += sb["G"]
        off16 += sb["G"] * 8  # 128 slots / 16
    CG = offG
    Gmax = max((sb["G"] for sb in struct), default=1)

    # per-edge slot assignment (per core)
    idx_all = np.zeros((C, 128, CG * 8), np.int16)
    dloc_all = np.full((C, 128, CG), -1.0, BF16)
    # precompute slot base for each (s,b,w): global slot start
    slot_base = np.zeros((NS, NBLK, W), np.int64)
    for sb in struct:
        s, b = sb["s"], sb["b"]
        for i, w in enumerate(range(sb["w_lo"], sb["w_hi"])):
            slot_base[s, b, w] = (sb["offG"] + sb["g_base"][i]) * 128

    for c in range(C):
        m = t_core == c
        k = keyW[m]
        order = np.argsort(k, kind="stable")
        ks = k[order]
        # rank within each run
        run_start = np.searchsorted(ks, np.arange(nkeys))
        rank = np.arange(len(ks)) - run_start[ks]
        sb_s = ks // (NBLK * W)
        sb_b = (ks // W) % NBLK
        sb_w = ks % W
        slot = slot_base[sb_s, sb_b, sb_w] + rank
        iv = idxv[m][order]
        dv = dloc[m][order]
        # idx wrapped layout: slot j -> (j%16, j//16), replicated x8
        prow = slot % 16
        pcol = slot // 16
        tmp = np.zeros((16, CG * 8), np.int16)
        tmp[prow, pcol] = iv.astype(np.int16)
        idx_all[c] = np.tile(tmp, (8, 1))
        dloc_all[c, slot % 128, slot // 128] = dv.astype(BF16)

    return dict(deg=deg, struct=struct, CG=CG, Gmax=Gmax, NS=NS, W=W,
                bounds=bounds, idx_all=idx_all, dloc_all=dloc_all)


def build_pool_meta(src, dst, batch, part, td_deg, bu_deg, cfg):
    """Layer-2-as-pooled-matmul coefficients.

    out_graph[g] = sum_v M[row(v), g] * hn2[v] + n_g * b2, where hn2 is the
    (AllGathered) dinv*(h1@W2) table in chunked table-row order.  M folds the
    edge aggregation (target-side dinv summed per source node and graph) and
    the self-loop diagonal."""
    C, N, G = cfg["N_CORES"], cfg["N"], cfg["NUM_GRAPHS"]
    gpc = G // C
    R = part["bounds"][-1]
    tr = part["table_row"]
    node_core = part["node_core"]
    batch = np.asarray(batch)

    dinv_td = 1.0 / np.sqrt(td_deg)          # [N] float64
    dinv_bu = 1.0 / np.sqrt(bu_deg)

    M_td = np.zeros((C, R, 128), np.float32)
    M_bu = np.zeros((C, R, 128), np.float32)
    # td: value row src, target dst -> coeff dinv_td[dst] into (core(dst), g(dst))
    c_t = node_core[dst]
    np.add.at(M_td, (c_t, tr[src], batch[dst] - c_t * gpc), dinv_td[dst].astype(np.float32))
    # bu: value row dst, target src -> coeff dinv_bu[src] into (core(src), g(src))
    c_s = node_core[src]
    np.add.at(M_bu, (c_s, tr[dst], batch[src] - c_s * gpc), dinv_bu[src].astype(np.float32))
    # self-loop diagonals: M[row(v), g(v)] += dinv[v] on the core owning v
    allv = np.arange(N)
    c_v = node_core[allv]
    np.add.at(M_td, (c_v, tr[allv], batch[allv] - c_v * gpc), dinv_td[allv].astype(np.float32))
    np.add.at(M_bu, (c_v, tr[allv], batch[allv] - c_v * gpc), dinv_bu[allv].astype(np.float32))
    n_g = np.bincount(batch, minlength=G).astype(np.float32)  # nodes per graph
    return dict(M_td=M_td.astype(BF16), M_bu=M_bu.astype(BF16), n_g=n_g, R=R)


def pool_batches(part, bw=32):
    """Window batches for the pooled-matmul phase, aligned to AG chunks."""
    bounds = part["bounds"]
    batches = []
    for q in range(len(bounds) - 1):
        u0 = bounds[q] // 128
        nwq = (bounds[q + 1] - bounds[q]) // 128
        for s in range(0, nwq, bw):
            batches.append((q, u0 + s, min(bw, nwq - s)))
    return batches


def pack_M(M, batches):
    """Repack [R, G] so each batch's block reads contiguously per partition:
    row p*nw+j holds window (u0+j) partition p."""
    out = np.empty_like(M)
    for (q, u0, nw) in batches:
        blk = M[u0 * 128:(u0 + nw) * 128].reshape(nw, 128, -1)
        out[u0 * 128:(u0 + nw) * 128] = blk.transpose(1, 0, 2).reshape(nw * 128, -1)
    return out


def build_all_inputs(x, edge_index, batch, Ws, bs, cfg):
    """Produce per-core in_maps plus structural metadata."""
    C = cfg["N_CORES"]
    N = cfg["N"]
    src = np.asarray(edge_index[0])
    dst = np.asarray(edge_index[1])
    part = build_partition(batch, cfg,
                           deg_td=np.bincount(dst, minlength=N),
                           deg_bu=np.bincount(src, minlength=N))
    NPC = part["NPC"]
    W = NPC // 128

    td = build_direction_meta(src, dst, part, cfg)   # gather src row, scatter to dst
    bu = build_direction_meta(dst, src, part, cfg)   # reversed
    pool = build_pool_meta(src, dst, batch, part, td["deg"], bu["deg"], cfg)
    batches = pool_batches(part, bw=16)
    pool["M_td"] = np.stack([pack_M(pool["M_td"][c], batches) for c in range(C)])
    pool["M_bu"] = np.stack([pack_M(pool["M_bu"][c], batches) for c in range(C)])

    Gmax = max(td["Gmax"], bu["Gmax"])
    iota_rep = np.tile(np.arange(128, dtype=np.float32), Gmax)[None, :].repeat(128, 0).astype(BF16)

    # per-core tensors
    in_maps = []
    xT_full = np.ascontiguousarray(np.asarray(x).T)  # [IN, N]
    batch_np = np.asarray(batch)
    ngb2 = np.concatenate([np.outer(pool["n_g"], bs[1]),
                           np.outer(pool["n_g"], bs[3])], axis=1).astype(np.float32)
    for c in range(C):
        lo, hi = part["starts"][c], part["starts"][c + 1]
        cnt = hi - lo
        li = part["node_local"][lo:hi]
        xT = np.zeros((cfg["IN_FEATS"], NPC), BF16)
        xT[:, li] = xT_full[:, lo:hi].astype(BF16)
        deg_t = np.ones((128, W), np.float32)
        deg_b = np.ones((128, W), np.float32)
        deg_t[li % 128, li // 128] = td["deg"][lo:hi].astype(np.float32)
        deg_b[li % 128, li // 128] = bu["deg"][lo:hi].astype(np.float32)
        im = dict(
            xT=xT, ident=np.eye(128, dtype=BF16),
            deg_td=deg_t, deg_bu=deg_b, iota_rep=iota_rep,
            M_td=pool["M_td"][c], M_bu=pool["M_bu"][c],
            ngb2=ngb2[c * part["gpc"]:(c + 1) * part["gpc"]],
            idx_td=td["idx_all"][c], idx_bu=bu["idx_all"][c],
            dstloc_td=td["dloc_all"][c], dstloc_bu=bu["dloc_all"][c],
            W_td1=Ws[0].astype(BF16), W_bu1=Ws[2].astype(BF16),
            W_td2=Ws[1].astype(BF16), W_bu2=Ws[3].astype(BF16),
            b_td1=np.tile(bs[0][None, :], (128, 1)).astype(np.float32),
            b_td2=np.tile(bs[1][None, :], (128, 1)).astype(np.float32),
            b_bu1=np.tile(bs[2][None, :], (128, 1)).astype(np.float32),
            b_bu2=np.tile(bs[3][None, :], (128, 1)).astype(np.float32),
        )
        in_maps.append(im)
    meta = dict(part=part, td=td, bu=bu, Gmax=Gmax, NPC=NPC, W=W, cfg=cfg,
                R=pool["R"], batches=batches)
    return in_maps, meta


# =====================================================================
# Bass program
# =====================================================================

def build_bass(meta):
    import concourse.bacc as bacc
    import concourse.mybir as mybir
    import concourse.tile as tile

    cfg = meta["cfg"]
    C = cfg["N_CORES"]
    NPC, W, Gmax = meta["NPC"], meta["W"], meta["Gmax"]
    IN, HID = cfg["IN_FEATS"], cfg["HIDDEN"]
    NBLK = cfg["NBLK"]
    f32, bf16, i16 = mybir.dt.float32, mybir.dt.bfloat16, mybir.dt.int16

    nc = bacc.Bacc("TRN2", target_bir_lowering=False, debug=False, num_devices=C,
                   num_swdge_queues=4)

    # ---- I/O ----
    ten = {}
    def inp(name, shape, dt):
        ten[name] = nc.dram_tensor(name, shape, dt, kind="ExternalInput")
        return ten[name]

    inp("xT", [IN, NPC], bf16)
    inp("deg_td", [128, W], f32); inp("deg_bu", [128, W], f32)
    inp("iota_rep", [128, Gmax * 128], bf16)
    inp("ident", [128, 128], bf16)
    inp("ngb2", [128, 2 * HID], f32)
    R = meta["R"]
    for d in ("td", "bu"):
        m = meta[d]
        inp(f"idx_{d}", [128, m["CG"] * 8], i16)
        inp(f"dstloc_{d}", [128, m["CG"]], bf16)
        inp(f"M_{d}", [R, 128], bf16)
        inp(f"W_{d}1", [IN, HID], bf16)
        inp(f"W_{d}2", [HID, HID], bf16)
        inp(f"b_{d}1", [128, HID], f32)
        inp(f"b_{d}2", [128, HID], f32)
    out_t = nc.dram_tensor("out", [128, 2 * HID], f32, kind="ExternalOutput")
    dbg = meta.get("dbg")
    if dbg:
        dbg_h1 = {d: nc.dram_tensor(f"dbg_h1_{d}", [NPC, HID], f32, kind="ExternalOutput")
                  for d in ("td", "bu")}
        dbg_m = {d: nc.dram_tensor(f"dbg_m_{d}", [NPC, HID], f32, kind="ExternalOutput")
                 for d in ("td", "bu")}

    # internal DRAM: AG inputs + tables
    ag_in, table = {}, {}
    for d in ("td", "bu"):
        for l in (1, 2):
            ag_in[d, l] = nc.dram_tensor(f"agin_{d}{l}", [NPC, HID], bf16, kind="Internal")
            table[d, l] = nc.dram_tensor(f"table_{d}{l}", [C * NPC, HID], bf16,
                                         kind="Internal", addr_space="Shared")

    rg = [list(range(C))]

    from contextlib import ExitStack
    with tile.TileContext(nc) as tc, ExitStack() as stack:
        def pool(name, bufs, space="SBUF"):
            return stack.enter_context(tc.tile_pool(name=name, bufs=bufs, space=space))

        const = pool("const", 1)
        xt_p = pool("xt", 6)
        hn_p = pool("hn", 4)                 # hn tiles to DRAM
        idx_p = pool("idx", 8)
        dl_p = pool("dl", 8)
        gat_p = pool("gat", 8)               # gathered edge tiles
        oh_p = pool("oh", 3)                 # one-hot tiles
        win_p = pool("win", 6, "PSUM")       # window psum, 4 windows/bank
        epi_p = pool("epi", 6)               # epilogue sbuf tiles
        h1_p = pool("h1", 4)
        t_p = pool("tt", 4)                  # transposes
        mb_p = pool("mb", 2)                 # pooled-matmul M batches
        tb_p = pool("tb", 2)                 # pooled-matmul hn2 batches
        outp = pool("outp", 1)
        hps_p = pool("hps", 1, "PSUM")
        pool_ps = pool("plps", 1, "PSUM")

        # ---- constants in SBUF ----
        iota = const.tile([128, Gmax * 128], bf16, tag="iota")
        nc.sync.dma_start(iota[:], ten["iota_rep"][:])
        Wt = {}
        for d in ("td", "bu"):
            for l, k in ((1, IN), (2, HID)):
                chunks = []
                for kk in range(k // 128):
                    t = const.tile([128, HID], bf16, tag=f"W_{d}{l}_{kk}", name=f"W_{d}{l}_{kk}")
                    nc.sync.dma_start(t[:], ten[f"W_{d}{l}"][kk * 128:(kk + 1) * 128, :])
                    chunks.append(t)
                Wt[d, l] = chunks
        Wcat = []
        for kk in range(IN // 128):
            t = const.tile([128, 2 * HID], bf16, tag=f"Wcat{kk}", name=f"Wcat{kk}")
            nc.sync.dma_start(t[:, 0:HID], ten["W_td1"][kk * 128:(kk + 1) * 128, :])
            nc.sync.dma_start(t[:, HID:2 * HID], ten["W_bu1"][kk * 128:(kk + 1) * 128, :])
            Wcat.append(t)
        bt = {}
        for d in ("td", "bu"):
            for l in (1, 2):
                t = const.tile([128, HID], f32, tag=f"b_{d}{l}", name=f"bt_{d}{l}")
                nc.sync.dma_start(t[:], ten[f"b_{d}{l}"][:])
                bt[d, l] = t
        zrow = const.tile([1, 512], bf16, tag="zrow")
        nc.gpsimd.memset(zrow[:], 0.0)
        ident = const.tile([128, 128], bf16, tag="ident")
        nc.sync.dma_start(ident[:], ten["ident"][:])
        ngb2_t = const.tile([128, 2 * HID], f32, tag="ngb2")
        nc.sync.dma_start(ngb2_t[:], ten["ngb2"][:])

        dinv = {}
        for d in ("td", "bu"):
            degt = const.tile([128, W], f32, tag=f"deg_{d}", name=f"degt_{d}")
            nc.sync.dma_start(degt[:], ten[f"deg_{d}"][:])
            rec = const.tile([128, W], f32, tag=f"rec_{d}", name=f"rec_{d}")
            nc.vector.reciprocal(rec[:], degt[:])
            dv = const.tile([128, W], f32, tag=f"dinv_{d}", name=f"dinv_{d}")
            nc.scalar.activation(dv[:], rec[:], mybir.ActivationFunctionType.Sqrt)
            dinv[d] = dv

        # ---- phase A1: conv1 tables (both directions share xT loads) ----
        cw = meta["part"]["cw"]
        bounds = meta["td"]["bounds"]

        def emit_ag(d, l, q):
            nc.gpsimd.collective_compute(
                "AllGather", mybir.AluOpType.bypass, replica_groups=rg,
                ins=[ag_in[d, l][128 * int(cw[q]):128 * int(cw[q + 1]), :]],
                outs=[table[d, l][bounds[q]:bounds[q + 1], :]])

        nK = IN // 128
        for q0 in range(NBLK):
            for w0 in range(int(cw[q0]), int(cw[q0 + 1]), 4):
                bwn = min(4, int(cw[q0 + 1]) - w0)
                xts = []
                for kk in range(nK):
                    t = xt_p.tile([128, 4 * 128], bf16, tag="xt", name=f"xt_{w0}_{kk}")
                    nc.sync.dma_start(t[:, :bwn * 128],
                                      ten["xT"][kk * 128:(kk + 1) * 128,
                                                w0 * 128:(w0 + bwn) * 128])
                    xts.append(t)
                hnb = {d: hn_p.tile([128, 4, HID], bf16, tag="hnb", name=f"hnb_{d}_{w0}")
                       for d in ("td", "bu")}
                for j in range(bwn):
                    w = w0 + j
                    hps = hps_p.tile([128, 2 * HID], f32, tag="hps")
                    for kk in range(nK):
                        nc.tensor.matmul(hps[:], xts[kk][:, j * 128:(j + 1) * 128],
                                         Wcat[kk][:], start=(kk == 0), stop=(kk == nK - 1))
                    for d, off in (("td", 0), ("bu", HID)):
                        nc.vector.tensor_scalar_mul(hnb[d][:, j, :], hps[:, off:off + HID],
                                                    dinv[d][:, w:w + 1])
                for d in ("td", "bu"):
                    nc.scalar.dma_start(
                        ag_in[d, 1][w0 * 128:(w0 + bwn) * 128, :]
                        .rearrange("(j p) f -> p j f", p=128),
                        hnb[d][:, :bwn, :])
            emit_ag("td", 1, q0)
            emit_ag("bu", 1, q0)

        # ---- edge phase for one conv ----
        def edge_phase(d, l):
            m = meta[d]
            first_mm = {}
            last_mm = {}
            # find last (sb_idx, group) per window for stop flags
            for sbi, sb in enumerate(m["struct"]):
                for i, w in enumerate(range(sb["w_lo"], sb["w_hi"])):
                    if sb["g_list"][i] > 0:
                        last_mm[w] = (sbi, int(sb["g_base"][i]) + int(sb["g_list"][i]) - 1)
            quad_tiles = {}
            def win_ap(w):
                q = w // 4
                if q not in quad_tiles:
                    qt = win_p.tile([128, 512], f32, tag="win",
                                    name=f"win_{d}{l}_{q}")
                    nc.tensor.matmul(qt[:], zrow[0:1, 0:128], zrow[0:1, 0:512],
                                     start=True, stop=False, skip_group_check=True)
                    quad_tiles[q] = qt
                return quad_tiles[q][:, (w % 4) * 128:(w % 4 + 1) * 128]
            for sbi, sb in enumerate(m["struct"]):
                G = sb["G"]
                if G == 0:
                    continue
                it = idx_p.tile([128, G * 8], i16, tag="idx")
                nc.sync.dma_start(it[:], ten[f"idx_{d}"][:, sb["off16"]:sb["off16"] + G * 8])
                dlt = dl_p.tile([128, G], bf16, tag="dl")
                nc.sync.dma_start(dlt[:], ten[f"dstloc_{d}"][:, sb["offG"]:sb["offG"] + G])
                gt = gat_p.tile([128, G, 128], bf16, tag="gat")
                blk = table[d, l][m["bounds"][sb["b"]]:m["bounds"][sb["b"] + 1], :]
                qn[0] += 1
                nc.gpsimd.dma_gather(gt[:], blk, it[:], num_idxs=G * 128,
                                     num_idxs_reg=G * 128, elem_size=HID,
                                     single_packet=False, queue_num=qn[0] % 4)
                oh = oh_p.tile([128, G * 128], bf16, tag="oh")
                nc.vector.tensor_tensor(
                    out=oh[:],
                    in0=dlt[:].rearrange("p (g o) -> p g o", o=1).to_broadcast([128, G, 128]),
                    in1=iota[:, :G * 128].rearrange("p (g f) -> p g f", f=128),
                    op=mybir.AluOpType.is_equal)
                for i, w in enumerate(range(sb["w_lo"], sb["w_hi"])):
                    gl = int(sb["g_list"][i])
                    if gl == 0:
                        continue
                    pt = win_ap(w)
                    gb = int(sb["g_base"][i])
                    for g in range(gb, gb + gl):
                        nc.tensor.matmul(
                            pt[:], oh[:, g * 128:(g + 1) * 128], gt[:, g, :],
                            start=False, stop=(last_mm[w] == (sbi, g)),
                            skip_group_check=True)
                # epilogues for completed supers: after last block of super
                if sb["b"] == NBLK - 1:
                    nsw = sb["w_hi"] - sb["w_lo"]
                    hnb = hn_p.tile([128, nsw, HID], bf16, tag="hn_ep")
                    nc.scalar.dma_start(
                        hnb[:], ag_in[d, l][sb["w_lo"] * 128:sb["w_hi"] * 128, :]
                        .rearrange("(j p) f -> p j f", p=128))
                    for i, w in enumerate(range(sb["w_lo"], sb["w_hi"])):
                        epilogue(d, l, w, win_ap(w), hnb[:, i, :])
                    quad_tiles.clear()
                    yield sb["w_hi"]
                else:
                    yield None

        def epilogue(d, l, w, pt, hn):
            o1 = epi_p.tile([128, HID], f32, tag="o1")
            nc.vector.scalar_tensor_tensor(
                out=o1[:], in0=pt[:], scalar=dinv[d][:, w:w + 1], in1=bt[d, l][:],
                op0=mybir.AluOpType.mult, op1=mybir.AluOpType.add)
            o2 = epi_p.tile([128, HID], bf16, tag="o2")
            nc.vector.scalar_tensor_tensor(
                out=o2[:], in0=hn, scalar=dinv[d][:, w:w + 1], in1=o1[:],
                op0=mybir.AluOpType.mult, op1=mybir.AluOpType.add)
            if dbg and l == 1:
                mf = epi_p.tile([128, HID], f32, tag="mf")
                nc.vector.tensor_copy(mf[:], pt[:])
                nc.sync.dma_start(dbg_m[d][w * 128:(w + 1) * 128, :], mf[:])
            h1 = h1_p.tile([128, HID], bf16, tag="h1")
            nc.scalar.activation(h1[:], o2[:], mybir.ActivationFunctionType.Relu)
            if dbg:
                h1f = epi_p.tile([128, HID], f32, tag="h1f")
                nc.vector.tensor_copy(h1f[:], h1[:])
                nc.sync.dma_start(dbg_h1[d][w * 128:(w + 1) * 128, :], h1f[:])
            tps = hps_p.tile([128, HID], bf16, tag="hps", name=f"tps_{d}_{w}")
            nc.tensor.transpose(tps[:], h1[:], ident[:])
            h1T = t_p.tile([128, HID], bf16, tag="h1T")
            nc.vector.tensor_copy(h1T[:], tps[:])
            h2 = hps_p.tile([128, HID], f32, tag="hps")
            nc.tensor.matmul(h2[:], h1T[:], Wt[d, 2][0][:], start=True, stop=True)
            hn2 = hn_p.tile([128, HID], bf16, tag="hn2")
            nc.vector.tensor_scalar_mul(hn2[:], h2[:], dinv[d][:, w:w + 1])
            nc.scalar.dma_start(ag_in[d, 2][w * 128:(w + 1) * 128, :], hn2[:])

        # ---- layer-2 pooled matmul: out[g] += M_u^T @ hn2_u per table window ----
        batches = meta["batches"]
        last_uq = {}   # d -> (q, u0, nw) of final batch
        for d in ("td", "bu"):
            last_uq[d] = batches[-1]

        def emit_pool(d, q):
            off = 0 if d == "td" else HID
            for (bq, u0, nw) in batches:
                if bq != q:
                    continue
                mt = mb_p.tile([128, nw, HID], bf16, tag="mb")
                nc.scalar.dma_start(
                    mt[:], ten[f"M_{d}"][u0 * 128:(u0 + nw) * 128, :]
                    .rearrange("(p j) g -> p j g", p=128))
                ht = tb_p.tile([128, nw, HID], bf16, tag="tb")
                nc.scalar.dma_start(
                    ht[:], table[d, 2][u0 * 128:(u0 + nw) * 128, :]
                    .rearrange("(j p) f -> p j f", p=128))
                for j in range(nw):
                    is_last = (bq, u0, nw) == last_uq[d] and j == nw - 1
                    nc.tensor.matmul(
                        pool_psum_t[:, off:off + HID],
                        mt[:, j, :], ht[:, j, :],
                        start=False, stop=is_last, skip_group_check=True)

        qn = [0]

        pool_psum_t = pool_ps.tile([128, 2 * HID], f32, tag="pool", name="pool_psum_t")
        nc.tensor.matmul(pool_psum_t[:], zrow[0:1, 0:128], zrow[0:1, 0:2 * HID],
                         start=True, stop=False, skip_group_check=True)

        def run_layer(l):
            gens = {"td": edge_phase("td", l), "bu": edge_phase("bu", l)}
            done = {"td": False, "bu": False}
            next_q = {"td": 0, "bu": 0}
            while not all(done.values()):
                for d in ("td", "bu"):
                    if done[d]:
                        continue
                    try:
                        res = next(gens[d])
                    except StopIteration:
                        done[d] = True
                        res = W
                    if l == 1 and res is not None:
                        while next_q[d] < NBLK and res >= int(cw[next_q[d] + 1]):
                            emit_ag(d, 2, next_q[d])
                            emit_pool(d, next_q[d])
                            next_q[d] += 1

        run_layer(1)

        outsb = outp.tile([128, 2 * HID], f32, tag="out")
        nc.vector.tensor_tensor(out=outsb[:], in0=pool_psum_t[:], in1=ngb2_t[:],
                                op=mybir.AluOpType.add)
        nc.sync.dma_start(out_t[:], outsb[:])

    nc.compile()
    return nc


# =====================================================================
# Entry point
# =====================================================================

def _run(inputs, cfg, trace=False):
    from concourse import bass_utils
    x = np.asarray(inputs["x"], np.float32)
    edge_index = np.asarray(inputs["edge_index"])
    batch = np.asarray(inputs["batch"])
    Ws = [np.asarray(inputs[k], np.float32) for k in ("W_td1", "W_td2", "W_bu1", "W_bu2")]
    bs = [np.asarray(inputs[k], np.float32) for k in ("b_td1", "b_td2", "b_bu1", "b_bu2")]
    in_maps, meta = build_all_inputs(x, edge_index, batch, Ws, bs, cfg)
    nc = build_bass(meta)
    res = bass_utils.run_bass_kernel_spmd(
        nc, in_maps, core_ids=list(range(cfg["N_CORES"])), trace=trace)
    gpc = meta["part"]["gpc"]
    out = np.concatenate([res.results[c]["out"][:gpc] for c in range(cfg["N_CORES"])], axis=0)
    return out.astype(np.float32), res


def kernel(**inputs):
    out, _ = _run(inputs, FULL_CFG, trace=False)
    return out



# revision 27
# speedup vs baseline: 1.8181x; 1.0758x over previous
"""BiGCN (2-layer bidirectional GCN + global add pool) on 8 Trainium2 NeuronCores.

Strategy (hardcoded for the nn_BiGCN_graphcl problem shapes):
  - Nodes are sharded graph-aligned: core c owns graphs [128c, 128c+128) and
    their (contiguous, batch-sorted) node range, padded to a common NPC.
  - Per direction (td / bu), edges are assigned to the core owning their
    target node.  GCNConv is computed as
        out = dinv * (scatter_add(hn[src], dst) + hn) + b,   hn = dinv * (x @ W)
    so no per-edge scaling is needed on device.
  - The hn table ([8*NPC, 128] bf16) is AllGathered between layers; each core
    gathers rows for its edge shard with dma_gather (256B rows), builds a
    staircase one-hot with a DVE is_equal against an iota constant, and
    segment-sums on the TensorEngine into per-window (128-node) PSUM tiles.
  - The SPMD program is identical on all cores: all per-core variation lives
    in uploaded index/data tensors; run lengths are padded to the max across
    cores (pad slots gather row 0 of the block and carry dstloc=-1 so their
    one-hot column is zero).
  - Graph pooling is a second one-hot matmul into a [128 graphs, 128] PSUM
    tile; the host just concatenates the 8 per-core [128, 256] outputs.
"""

import math
import numpy as np
import ml_dtypes

BF16 = ml_dtypes.bfloat16

# ---------------------------------------------------------------- problem cfg
FULL_CFG = dict(
    N=100000, E=1600000, IN_FEATS=256, HIDDEN=128, OUT_FEATS=128,
    NUM_GRAPHS=1024, N_CORES=8, SW=8, NBLK=4,
)


def _round_up(x, m):
    return (x + m - 1) // m * m


# =====================================================================
# Host-side metadata construction
# =====================================================================

def build_partition(batch, cfg, deg_td=None, deg_bu=None):
    """Graph-aligned node partition. Returns dict with per-core node ranges.

    If degree arrays are given, each core's local node order is permuted so
    that per-window (128-node) degree sums cluster just under multiples of
    4*128 edges per (window, src-block) run, minimizing ceil-128 padding."""
    N, C, G = cfg["N"], cfg["N_CORES"], cfg["NUM_GRAPHS"]
    gpc = G // C  # graphs per core
    starts = np.searchsorted(batch, np.arange(0, G + 1, gpc))
    counts = np.diff(starts)
    NPC = max(128, _round_up(int(counts.max()), 128))
    W = NPC // 128
    node_core = np.searchsorted(starts[1:], np.arange(N), side="right")
    node_local = np.arange(N) - starts[node_core]

    if deg_td is not None:
        NBLK = cfg["NBLK"]
        MARGIN = 45 * NBLK  # leave room for cross-core/block-split variance
        for c in range(C):
            lo, hi = starts[c], starts[c + 1]
            cnt = hi - lo
            dt = deg_td[lo:hi].astype(np.int64)
            db = deg_bu[lo:hi].astype(np.int64)
            order = np.argsort(-(dt + db), kind="stable")
            tg_t = np.full(W, dt.sum() / W)
            tg_b = np.full(W, db.sum() / W)
            rem_t = tg_t.astype(np.float64).copy()
            rem_b = tg_b.astype(np.float64).copy()
            room = np.full(W, 128, np.int64)
            assign = np.empty(cnt, np.int64)
            for j in order:
                score = np.minimum(rem_t - dt[j], rem_b - db[j])
                score[room <= 0] = -np.inf
                w = int(np.argmax(score))
                assign[j] = w
                rem_t[w] -= dt[j]
                rem_b[w] -= db[j]
                room[w] -= 1
            # positions: window-major order
            slot_in_w = np.zeros(W, np.int64)
            newloc = np.empty(cnt, np.int64)
            for j in range(cnt):
                w = assign[j]
                newloc[j] = w * 128 + slot_in_w[w]
                slot_in_w[w] += 1
            node_local[lo:hi] = newloc

    # ---- chunk decomposition: 4 window-chunks, sized so per-(window, chunk)
    # gather runs land just under multiples of 128, and each chunk's block of
    # 8*128*w_q table rows stays within int16 index range. ----
    NBLK = cfg["NBLK"]
    mean_w = max(1.0, (deg_td.sum() + deg_bu.sum()) / (2.0 * C * W)) if deg_td is not None else 128.0
    wmax = min(W, (32767 // (128 * C)))

    def padfrac(wb):
        r = wb / W * mean_w  # mean edges per (window, this-chunk) run
        if r <= 0:
            return 0.0
        margin = 1.6 * np.sqrt(r) + 6
        gslots = 128 * np.ceil((r + margin) / 128)
        return (gslots - r) * 1.0

    best = None
    for w1 in range(1, wmax + 1):
        for w2 in range(w1, wmax + 1):
            for w3 in range(w2, wmax + 1):
                w4 = W - w1 - w2 - w3
                if w4 < w3 or w4 > wmax:
                    continue
                cost = padfrac(w1) + padfrac(w2) + padfrac(w3) + padfrac(w4)
                if best is None or cost < best[0]:
                    best = (cost, (w1, w2, w3, w4))
    ws = sorted(best[1], reverse=True) if best else [W]
    # big chunks first: their AG starts earliest, and the LAST chunk (whose
    # pooled-matmul work forms the serial tail) is smallest
    cw = np.concatenate([[0], np.cumsum(ws)])
    assert cw[-1] == W

    chunk_of_w = np.searchsorted(cw[1:], np.arange(W), side="right")
    q = chunk_of_w[np.minimum(node_local // 128, W - 1)]
    rpr = 128 * np.diff(cw)  # rows per rank per chunk
    base = np.concatenate([[0], np.cumsum(rpr * C)])
    table_row = base[q] + node_core * rpr[q] + (node_local - 128 * cw[q])
    bounds = [int(b) for b in base]
    return dict(starts=starts, counts=counts, NPC=NPC, gpc=gpc,
                node_core=node_core.astype(np.int64),
                node_local=node_local.astype(np.int64),
                table_row=table_row.astype(np.int64),
                cw=cw, bounds=bounds)


def build_direction_meta(gather_nodes, target_nodes, part, cfg):
    """Build per-core gather index / dstloc arrays and the uniform group
    structure for one edge direction.

    gather_nodes[e]: node whose table row is gathered for edge e.
    target_nodes[e]: node receiving the contribution.
    """
    N, C = cfg["N"], cfg["N_CORES"]
    SW, NBLK = cfg["SW"], cfg["NBLK"]
    NPC = part["NPC"]
    W = NPC // 128
    NS = (W + SW - 1) // SW
    R = C * NPC

    deg = np.bincount(target_nodes, minlength=N).astype(np.float64) + 1.0

    bounds = part["bounds"]
    assert len(bounds) == NBLK + 1
    assert all(bounds[i + 1] - bounds[i] <= 32767 for i in range(NBLK))
    bounds_arr = np.array(bounds[1:-1])

    tr_g = part["table_row"][gather_nodes]
    t_core = part["node_core"][target_nodes]
    t_local = part["node_local"][target_nodes]
    lw = t_local // 128          # window
    dloc = t_local % 128         # position within window
    blk = np.searchsorted(bounds_arr, tr_g, side="right")
    idxv = tr_g - np.array(bounds[:-1])[blk]
    sup = lw // SW

    # per (core, s, b, w) counts -> uniform G
    keyW = (sup * NBLK + blk) * W + lw  # key within a core
    nkeys = NS * NBLK * W
    counts = np.zeros((C, nkeys), np.int64)
    for c in range(C):
        m = t_core == c
        counts[c] = np.bincount(keyW[m], minlength=nkeys)
    max_counts = counts.max(axis=0).reshape(NS, NBLK, W)

    G = np.ceil(max_counts / 128).astype(np.int64)  # groups per (s,b,w)
    # ensure every window has at least one group (psum must be written)
    for s in range(NS):
        w_lo, w_hi = s * SW, min((s + 1) * SW, W)
        for w in range(w_lo, w_hi):
            if G[s, :, w].sum() == 0:
                G[s, 0, w] = 1
        G[s, :, :w_lo] = 0
        G[s, :, w_hi:] = 0

    # structure: per (s,b): window col bases, totals
    struct = []
    for s in range(NS):
        w_lo, w_hi = s * SW, min((s + 1) * SW, W)
        for b in range(NBLK):
            g_list = G[s, b, w_lo:w_hi]
            base = np.concatenate([[0], np.cumsum(g_list)])
            struct.append(dict(s=s, b=b, w_lo=w_lo, w_hi=w_hi,
                               g_list=g_list, g_base=base,
                               G=int(g_list.sum())))
    # global column offsets
    offG = 0
    off16 = 0
    for sb in struct:
        sb["offG"] = offG
        sb["off16"] = off16
        offG += sb["G"]
        off16 += sb["G"] * 8  # 128 slots / 16
    CG = offG
    Gmax = max((sb["G"] for sb in struct), default=1)

    # per-edge slot assignment (per core)
    idx_all = np.zeros((C, 128, CG * 8), np.int16)
    dloc_all = np.full((C, 128, CG), -1.0, BF16)
    # precompute slot base for each (s,b,w): global slot start
    slot_base = np.zeros((NS, NBLK, W), np.int64)
    for sb in struct:
        s, b = sb["s"], sb["b"]
        for i, w in enumerate(range(sb["w_lo"], sb["w_hi"])):
            slot_base[s, b, w] = (sb["offG"] + sb["g_base"][i]) * 128

    for c in range(C):
        m = t_core == c
        k = keyW[m]
        order = np.argsort(k, kind="stable")
        ks = k[order]
        # rank within each run
        run_start = np.searchsorted(ks, np.arange(nkeys))
        rank = np.arange(len(ks)) - run_start[ks]
        sb_s = ks // (NBLK * W)
        sb_b = (ks // W) % NBLK
        sb_w = ks % W
        slot = slot_base[sb_s, sb_b, sb_w] + rank
        iv = idxv[m][order]
        dv = dloc[m][order]
        # idx wrapped layout: slot j -> (j%16, j//16), replicated x8
        prow = slot % 16
        pcol = slot // 16
        tmp = np.zeros((16, CG * 8), np.int16)
        tmp[prow, pcol] = iv.astype(np.int16)
        idx_all[c] = np.tile(tmp, (8, 1))
        dloc_all[c, slot % 128, slot // 128] = dv.astype(BF16)

    return dict(deg=deg, struct=struct, CG=CG, Gmax=Gmax, NS=NS, W=W,
                bounds=bounds, idx_all=idx_all, dloc_all=dloc_all)


def build_pool_meta(src, dst, batch, part, td_deg, bu_deg, cfg):
    """Layer-2-as-pooled-matmul coefficients.

    out_graph[g] = sum_v M[row(v), g] * hn2[v] + n_g * b2, where hn2 is the
    (AllGathered) dinv*(h1@W2) table in chunked table-row order.  M folds the
    edge aggregation (target-side dinv summed per source node and graph) and
    the self-loop diagonal."""
    C, N, G = cfg["N_CORES"], cfg["N"], cfg["NUM_GRAPHS"]
    gpc = G // C
    R = part["bounds"][-1]
    tr = part["table_row"]
    node_core = part["node_core"]
    batch = np.asarray(batch)

    dinv_td = 1.0 / np.sqrt(td_deg)          # [N] float64
    dinv_bu = 1.0 / np.sqrt(bu_deg)

    M_td = np.zeros((C, R, 128), np.float32)
    M_bu = np.zeros((C, R, 128), np.float32)
    # td: value row src, target dst -> coeff dinv_td[dst] into (core(dst), g(dst))
    c_t = node_core[dst]
    np.add.at(M_td, (c_t, tr[src], batch[dst] - c_t * gpc), dinv_td[dst].astype(np.float32))
    # bu: value row dst, target src -> coeff dinv_bu[src] into (core(src), g(src))
    c_s = node_core[src]
    np.add.at(M_bu, (c_s, tr[dst], batch[src] - c_s * gpc), dinv_bu[src].astype(np.float32))
    # self-loop diagonals: M[row(v), g(v)] += dinv[v] on the core owning v
    allv = np.arange(N)
    c_v = node_core[allv]
    np.add.at(M_td, (c_v, tr[allv], batch[allv] - c_v * gpc), dinv_td[allv].astype(np.float32))
    np.add.at(M_bu, (c_v, tr[allv], batch[allv] - c_v * gpc), dinv_bu[allv].astype(np.float32))
    n_g = np.bincount(batch, minlength=G).astype(np.float32)  # nodes per graph
    return dict(M_td=M_td.astype(BF16), M_bu=M_bu.astype(BF16), n_g=n_g, R=R)


def pool_batches(part, bw=32):
    """Window batches for the pooled-matmul phase, aligned to AG chunks."""
    bounds = part["bounds"]
    batches = []
    for q in range(len(bounds) - 1):
        u0 = bounds[q] // 128
        nwq = (bounds[q + 1] - bounds[q]) // 128
        for s in range(0, nwq, bw):
            batches.append((q, u0 + s, min(bw, nwq - s)))
    return batches


def pack_M(M, batches):
    """Repack [R, G] so each batch's block reads contiguously per partition:
    row p*nw+j holds window (u0+j) partition p."""
    out = np.empty_like(M)
    for (q, u0, nw) in batches:
        blk = M[u0 * 128:(u0 + nw) * 128].reshape(nw, 128, -1)
        out[u0 * 128:(u0 + nw) * 128] = blk.transpose(1, 0, 2).reshape(nw * 128, -1)
    return out


def build_all_inputs(x, edge_index, batch, Ws, bs, cfg):
    """Produce per-core in_maps plus structural metadata."""
    C = cfg["N_CORES"]
    N = cfg["N"]
    src = np.asarray(edge_index[0])
    dst = np.asarray(edge_index[1])
    part = build_partition(batch, cfg,
                           deg_td=np.bincount(dst, minlength=N),
                           deg_bu=np.bincount(src, minlength=N))
    NPC = part["NPC"]
    W = NPC // 128

    td = build_direction_meta(src, dst, part, cfg)   # gather src row, scatter to dst
    bu = build_direction_meta(dst, src, part, cfg)   # reversed
    pool = build_pool_meta(src, dst, batch, part, td["deg"], bu["deg"], cfg)
    batches = pool_batches(part, bw=16)
    pool["M_td"] = np.stack([pack_M(pool["M_td"][c], batches) for c in range(C)])
    pool["M_bu"] = np.stack([pack_M(pool["M_bu"][c], batches) for c in range(C)])

    Gmax = max(td["Gmax"], bu["Gmax"])
    iota_rep = np.tile(np.arange(128, dtype=np.float32), Gmax)[None, :].repeat(128, 0).astype(BF16)

    # per-core tensors
    in_maps = []
    xT_full = np.ascontiguousarray(np.asarray(x).T)  # [IN, N]
    batch_np = np.asarray(batch)
    ngb2 = np.concatenate([np.outer(pool["n_g"], bs[1]),
                           np.outer(pool["n_g"], bs[3])], axis=1).astype(np.float32)
    for c in range(C):
        lo, hi = part["starts"][c], part["starts"][c + 1]
        cnt = hi - lo
        li = part["node_local"][lo:hi]
        xT = np.zeros((cfg["IN_FEATS"], NPC), BF16)
        xT[:, li] = xT_full[:, lo:hi].astype(BF16)
        deg_t = np.ones((128, W), np.float32)
        deg_b = np.ones((128, W), np.float32)
        deg_t[li % 128, li // 128] = td["deg"][lo:hi].astype(np.float32)
        deg_b[li % 128, li // 128] = bu["deg"][lo:hi].astype(np.float32)
        im = dict(
            xT=xT, ident=np.eye(128, dtype=BF16),
            deg_td=deg_t, deg_bu=deg_b, iota_rep=iota_rep,
            M_td=pool["M_td"][c], M_bu=pool["M_bu"][c],
            ngb2=ngb2[c * part["gpc"]:(c + 1) * part["gpc"]],
            idx_td=td["idx_all"][c], idx_bu=bu["idx_all"][c],
            dstloc_td=td["dloc_all"][c], dstloc_bu=bu["dloc_all"][c],
            W_td1=Ws[0].astype(BF16), W_bu1=Ws[2].astype(BF16),
            W_td2=Ws[1].astype(BF16), W_bu2=Ws[3].astype(BF16),
            b_td1=np.tile(bs[0][None, :], (128, 1)).astype(np.float32),
            b_td2=np.tile(bs[1][None, :], (128, 1)).astype(np.float32),
            b_bu1=np.tile(bs[2][None, :], (128, 1)).astype(np.float32),
            b_bu2=np.tile(bs[3][None, :], (128, 1)).astype(np.float32),
        )
        in_maps.append(im)
    meta = dict(part=part, td=td, bu=bu, Gmax=Gmax, NPC=NPC, W=W, cfg=cfg,
                R=pool["R"], batches=batches)
    return in_maps, meta


# =====================================================================
# Bass program
# =====================================================================

def build_bass(meta):
    import concourse.bacc as bacc
    import concourse.mybir as mybir
    import concourse.tile as tile

    cfg = meta["cfg"]
    C = cfg["N_CORES"]
    NPC, W, Gmax = meta["NPC"], meta["W"], meta["Gmax"]
    IN, HID = cfg["IN_FEATS"], cfg["HIDDEN"]
    NBLK = cfg["NBLK"]
    f32, bf16, i16 = mybir.dt.float32, mybir.dt.bfloat16, mybir.dt.int16

    nc = bacc.Bacc("TRN2", target_bir_lowering=False, debug=False, num_devices=C,
                   num_swdge_queues=4)

    # ---- I/O ----
    ten = {}
    def inp(name, shape, dt):
        ten[name] = nc.dram_tensor(name, shape, dt, kind="ExternalInput")
        return ten[name]

    inp("xT", [IN, NPC], bf16)
    inp("deg_td", [128, W], f32); inp("deg_bu", [128, W], f32)
    inp("iota_rep", [128, Gmax * 128], bf16)
    inp("ident", [128, 128], bf16)
    inp("ngb2", [128, 2 * HID], f32)
    R = meta["R"]
    for d in ("td", "bu"):
        m = meta[d]
        inp(f"idx_{d}", [128, m["CG"] * 8], i16)
        inp(f"dstloc_{d}", [128, m["CG"]], bf16)
        inp(f"M_{d}", [R, 128], bf16)
        inp(f"W_{d}1", [IN, HID], bf16)
        inp(f"W_{d}2", [HID, HID], bf16)
        inp(f"b_{d}1", [128, HID], f32)
        inp(f"b_{d}2", [128, HID], f32)
    out_t = nc.dram_tensor("out", [128, 2 * HID], f32, kind="ExternalOutput")
    dbg = meta.get("dbg")
    if dbg:
        dbg_h1 = {d: nc.dram_tensor(f"dbg_h1_{d}", [NPC, HID], f32, kind="ExternalOutput")
                  for d in ("td", "bu")}
        dbg_m = {d: nc.dram_tensor(f"dbg_m_{d}", [NPC, HID], f32, kind="ExternalOutput")
                 for d in ("td", "bu")}

    # internal DRAM: AG inputs + tables
    ag_in, table = {}, {}
    for d in ("td", "bu"):
        for l in (1, 2):
            ag_in[d, l] = nc.dram_tensor(f"agin_{d}{l}", [NPC, HID], bf16, kind="Internal")
            table[d, l] = nc.dram_tensor(f"table_{d}{l}", [C * NPC, HID], bf16,
                                         kind="Internal", addr_space="Shared")

    rg = [list(range(C))]

    from contextlib import ExitStack
    with tile.TileContext(nc) as tc, ExitStack() as stack:
        def pool(name, bufs, space="SBUF"):
            return stack.enter_context(tc.tile_pool(name=name, bufs=bufs, space=space))

        const = pool("const", 1)
        xt_p = pool("xt", 6)
        hna1_p = pool("hna1", 3)             # A1 hn batches
        hnep_p = pool("hnep", 3)             # epilogue hn reloads
        hn2_p = pool("hn2", 3)               # epilogue hn2 store batches
        idx_p = pool("idx", 3)
        dl_p = pool("dl", 3)
        gat_p = pool("gat", 8)               # gathered edge tiles
        oh_p = pool("oh", 3)                 # one-hot tiles
        win_p = pool("win", 6, "PSUM")       # window psum, 4 windows/bank
        epi_p = pool("epi", 6)               # epilogue sbuf tiles
        h1_p = pool("h1", 4)
        t_p = pool("tt", 4)                  # transposes
        mb_p = pool("mb", 2)                 # pooled-matmul M batches
        tb_p = pool("tb", 2)                 # pooled-matmul hn2 batches
        outp = pool("outp", 1)
        hps_p = pool("hps", 1, "PSUM")
        pool_ps = pool("plps", 1, "PSUM")

        # ---- constants in SBUF ----
        iota = const.tile([128, Gmax * 128], bf16, tag="iota")
        nc.sync.dma_start(iota[:], ten["iota_rep"][:])
        Wt = {}
        for d in ("td", "bu"):
            for l, k in ((1, IN), (2, HID)):
                chunks = []
                for kk in range(k // 128):
                    t = const.tile([128, HID], bf16, tag=f"W_{d}{l}_{kk}", name=f"W_{d}{l}_{kk}")
                    nc.sync.dma_start(t[:], ten[f"W_{d}{l}"][kk * 128:(kk + 1) * 128, :])
                    chunks.append(t)
                Wt[d, l] = chunks
        Wcat = []
        for kk in range(IN // 128):
            t = const.tile([128, 2 * HID], bf16, tag=f"Wcat{kk}", name=f"Wcat{kk}")
            nc.sync.dma_start(t[:, 0:HID], ten["W_td1"][kk * 128:(kk + 1) * 128, :])
            nc.sync.dma_start(t[:, HID:2 * HID], ten["W_bu1"][kk * 128:(kk + 1) * 128, :])
            Wcat.append(t)
        bt = {}
        for d in ("td", "bu"):
            for l in (1, 2):
                t = const.tile([128, HID], f32, tag=f"b_{d}{l}", name=f"bt_{d}{l}")
                nc.sync.dma_start(t[:], ten[f"b_{d}{l}"][:])
                bt[d, l] = t
        zrow = const.tile([1, 512], bf16, tag="zrow")
        nc.gpsimd.memset(zrow[:], 0.0)
        ident = const.tile([128, 128], bf16, tag="ident")
        nc.sync.dma_start(ident[:], ten["ident"][:])
        ngb2_t = const.tile([128, 2 * HID], f32, tag="ngb2")
        nc.sync.dma_start(ngb2_t[:], ten["ngb2"][:])

        dinv = {}
        for d in ("td", "bu"):
            degt = const.tile([128, W], f32, tag=f"deg_{d}", name=f"degt_{d}")
            nc.sync.dma_start(degt[:], ten[f"deg_{d}"][:])
            rec = const.tile([128, W], f32, tag=f"rec_{d}", name=f"rec_{d}")
            nc.vector.reciprocal(rec[:], degt[:])
            dv = const.tile([128, W], f32, tag=f"dinv_{d}", name=f"dinv_{d}")
            nc.scalar.activation(dv[:], rec[:], mybir.ActivationFunctionType.Sqrt)
            dinv[d] = dv

        # ---- phase A1: conv1 tables (both directions share xT loads) ----
        cw = meta["part"]["cw"]
        bounds = meta["td"]["bounds"]

        def emit_ag(d, l, q):
            nc.gpsimd.collective_compute(
                "AllGather", mybir.AluOpType.bypass, replica_groups=rg,
                ins=[ag_in[d, l][128 * int(cw[q]):128 * int(cw[q + 1]), :]],
                outs=[table[d, l][bounds[q]:bounds[q + 1], :]])

        nK = IN // 128
        for q0 in range(NBLK):
            for w0 in range(int(cw[q0]), int(cw[q0 + 1]), 4):
                bwn = min(4, int(cw[q0 + 1]) - w0)
                xts = []
                for kk in range(nK):
                    t = xt_p.tile([128, 4 * 128], bf16, tag="xt", name=f"xt_{w0}_{kk}")
                    nc.sync.dma_start(t[:, :bwn * 128],
                                      ten["xT"][kk * 128:(kk + 1) * 128,
                                                w0 * 128:(w0 + bwn) * 128])
                    xts.append(t)
                hnb = {d: hna1_p.tile([128, 4, HID], bf16, tag="hnb", name=f"hnb_{d}_{w0}")
                       for d in ("td", "bu")}
                for j in range(bwn):
                    w = w0 + j
                    hps = hps_p.tile([128, 2 * HID], f32, tag="hps")
                    for kk in range(nK):
                        nc.tensor.matmul(hps[:], xts[kk][:, j * 128:(j + 1) * 128],
                                         Wcat[kk][:], start=(kk == 0), stop=(kk == nK - 1))
                    for d, off in (("td", 0), ("bu", HID)):
                        nc.vector.tensor_scalar_mul(hnb[d][:, j, :], hps[:, off:off + HID],
                                                    dinv[d][:, w:w + 1])
                for d in ("td", "bu"):
                    nc.scalar.dma_start(
                        ag_in[d, 1][w0 * 128:(w0 + bwn) * 128, :]
                        .rearrange("(j p) f -> p j f", p=128),
                        hnb[d][:, :bwn, :])
            emit_ag("td", 1, q0)
            emit_ag("bu", 1, q0)

        # ---- edge phase for one conv ----
        def edge_phase(d, l):
            m = meta[d]
            first_mm = {}
            last_mm = {}
            # find last (sb_idx, group) per window for stop flags
            for sbi, sb in enumerate(m["struct"]):
                for i, w in enumerate(range(sb["w_lo"], sb["w_hi"])):
                    if sb["g_list"][i] > 0:
                        last_mm[w] = (sbi, int(sb["g_base"][i]) + int(sb["g_list"][i]) - 1)
            quad_tiles = {}
            def win_ap(w):
                q = w // 4
                if q not in quad_tiles:
                    qt = win_p.tile([128, 512], f32, tag="win",
                                    name=f"win_{d}{l}_{q}")
                    nc.tensor.matmul(qt[:], zrow[0:1, 0:128], zrow[0:1, 0:512],
                                     start=True, stop=False, skip_group_check=True)
                    quad_tiles[q] = qt
                return quad_tiles[q][:, (w % 4) * 128:(w % 4 + 1) * 128]
            structs = m["struct"]
            it_sup = dlt_sup = None
            sup_off16 = sup_offG = 0
            for sbi, sb in enumerate(structs):
                if sbi % NBLK == 0:
                    supG = sum(x["G"] for x in structs[sbi:sbi + NBLK])
                    sup_off16, sup_offG = sb["off16"], sb["offG"]
                    if supG > 0:
                        it_sup = idx_p.tile([128, supG * 8], i16, tag="idx")
                        nc.sync.dma_start(
                            it_sup[:], ten[f"idx_{d}"][:, sup_off16:sup_off16 + supG * 8])
                        dlt_sup = dl_p.tile([128, supG], bf16, tag="dl")
                        nc.sync.dma_start(
                            dlt_sup[:], ten[f"dstloc_{d}"][:, sup_offG:sup_offG + supG])
                G = sb["G"]
                if G == 0:
                    continue
                r16 = sb["off16"] - sup_off16
                rG = sb["offG"] - sup_offG
                gt = gat_p.tile([128, G, 128], bf16, tag="gat")
                blk = table[d, l][m["bounds"][sb["b"]]:m["bounds"][sb["b"] + 1], :]
                qn[0] += 1
                nc.gpsimd.dma_gather(gt[:], blk, it_sup[:, r16:r16 + G * 8],
                                     num_idxs=G * 128,
                                     num_idxs_reg=G * 128, elem_size=HID,
                                     single_packet=False, queue_num=qn[0] % 4)
                oh = oh_p.tile([128, G * 128], bf16, tag="oh")
                nc.vector.tensor_tensor(
                    out=oh[:],
                    in0=dlt_sup[:, rG:rG + G].rearrange("p (g o) -> p g o", o=1)
                    .to_broadcast([128, G, 128]),
                    in1=iota[:, :G * 128].rearrange("p (g f) -> p g f", f=128),
                    op=mybir.AluOpType.is_equal)
                for i, w in enumerate(range(sb["w_lo"], sb["w_hi"])):
                    gl = int(sb["g_list"][i])
                    if gl == 0:
                        continue
                    pt = win_ap(w)
                    gb = int(sb["g_base"][i])
                    for g in range(gb, gb + gl):
                        nc.tensor.matmul(
                            pt[:], oh[:, g * 128:(g + 1) * 128], gt[:, g, :],
                            start=False, stop=(last_mm[w] == (sbi, g)),
                            skip_group_check=True)
                # epilogues for completed supers: after last block of super
                if sb["b"] == NBLK - 1:
                    nsw = sb["w_hi"] - sb["w_lo"]
                    hnb = hnep_p.tile([128, nsw, HID], bf16, tag="hn_ep")
                    nc.scalar.dma_start(
                        hnb[:], ag_in[d, l][sb["w_lo"] * 128:sb["w_hi"] * 128, :]
                        .rearrange("(j p) f -> p j f", p=128))
                    hn2b = hn2_p.tile([128, nsw, HID], bf16, tag="hn2b")
                    for i, w in enumerate(range(sb["w_lo"], sb["w_hi"])):
                        epilogue(d, l, w, win_ap(w), hnb[:, i, :], hn2b[:, i, :])
                    nc.scalar.dma_start(
                        ag_in[d, 2][sb["w_lo"] * 128:sb["w_hi"] * 128, :]
                        .rearrange("(j p) f -> p j f", p=128),
                        hn2b[:])
                    quad_tiles.clear()
                    yield sb["w_hi"]
                else:
                    yield None

        def epilogue(d, l, w, pt, hn, hn2_out):
            o1 = epi_p.tile([128, HID], f32, tag="o1")
            nc.vector.scalar_tensor_tensor(
                out=o1[:], in0=pt[:], scalar=dinv[d][:, w:w + 1], in1=bt[d, l][:],
                op0=mybir.AluOpType.mult, op1=mybir.AluOpType.add)
            o2 = epi_p.tile([128, HID], bf16, tag="o2")
            nc.vector.scalar_tensor_tensor(
                out=o2[:], in0=hn, scalar=dinv[d][:, w:w + 1], in1=o1[:],
                op0=mybir.AluOpType.mult, op1=mybir.AluOpType.add)
            if dbg and l == 1:
                mf = epi_p.tile([128, HID], f32, tag="mf")
                nc.vector.tensor_copy(mf[:], pt[:])
                nc.sync.dma_start(dbg_m[d][w * 128:(w + 1) * 128, :], mf[:])
            h1 = h1_p.tile([128, HID], bf16, tag="h1")
            nc.scalar.activation(h1[:], o2[:], mybir.ActivationFunctionType.Relu)
            if dbg:
                h1f = epi_p.tile([128, HID], f32, tag="h1f")
                nc.vector.tensor_copy(h1f[:], h1[:])
                nc.sync.dma_start(dbg_h1[d][w * 128:(w + 1) * 128, :], h1f[:])
            tps = hps_p.tile([128, HID], bf16, tag="hps", name=f"tps_{d}_{w}")
            nc.tensor.transpose(tps[:], h1[:], ident[:])
            h1T = t_p.tile([128, HID], bf16, tag="h1T")
            nc.vector.tensor_copy(h1T[:], tps[:])
            h2 = hps_p.tile([128, HID], f32, tag="hps")
            nc.tensor.matmul(h2[:], h1T[:], Wt[d, 2][0][:], start=True, stop=True)
            nc.vector.tensor_scalar_mul(hn2_out, h2[:], dinv[d][:, w:w + 1])

        # ---- layer-2 pooled matmul: out[g] += M_u^T @ hn2_u per table window ----
        batches = meta["batches"]
        last_uq = {}   # d -> (q, u0, nw) of final batch
        for d in ("td", "bu"):
            last_uq[d] = batches[-1]

        def emit_pool(d, q):
            off = 0 if d == "td" else HID
            for (bq, u0, nw) in batches:
                if bq != q:
                    continue
                mt = mb_p.tile([128, nw, HID], bf16, tag="mb")
                nc.scalar.dma_start(
                    mt[:], ten[f"M_{d}"][u0 * 128:(u0 + nw) * 128, :]
                    .rearrange("(p j) g -> p j g", p=128))
                ht = tb_p.tile([128, nw, HID], bf16, tag="tb")
                nc.scalar.dma_start(
                    ht[:], table[d, 2][u0 * 128:(u0 + nw) * 128, :]
                    .rearrange("(j p) f -> p j f", p=128))
                for j in range(nw):
                    is_last = (bq, u0, nw) == last_uq[d] and j == nw - 1
                    nc.tensor.matmul(
                        pool_psum_t[:, off:off + HID],
                        mt[:, j, :], ht[:, j, :],
                        start=False, stop=is_last, skip_group_check=True)

        qn = [0]

        pool_psum_t = pool_ps.tile([128, 2 * HID], f32, tag="pool", name="pool_psum_t")
        nc.tensor.matmul(pool_psum_t[:], zrow[0:1, 0:128], zrow[0:1, 0:2 * HID],
                         start=True, stop=False, skip_group_check=True)

        def run_layer(l):
            gens = {"td": edge_phase("td", l), "bu": edge_phase("bu", l)}
            done = {"td": False, "bu": False}
            next_q = {"td": 0, "bu": 0}
            while not all(done.values()):
                for d in ("td", "bu"):
                    if done[d]:
                        continue
                    try:
                        res = next(gens[d])
                    except StopIteration:
                        done[d] = True
                        res = W
                    if l == 1 and res is not None:
                        while next_q[d] < NBLK and res >= int(cw[next_q[d] + 1]):
                            emit_ag(d, 2, next_q[d])
                            emit_pool(d, next_q[d])
                            next_q[d] += 1

        run_layer(1)

        outsb = outp.tile([128, 2 * HID], f32, tag="out")
        nc.vector.tensor_tensor(out=outsb[:], in0=pool_psum_t[:], in1=ngb2_t[:],
                                op=mybir.AluOpType.add)
        nc.sync.dma_start(out_t[:], outsb[:])

    nc.compile()
    return nc


# =====================================================================
# Entry point
# =====================================================================

def _run(inputs, cfg, trace=False):
    from concourse import bass_utils
    x = np.asarray(inputs["x"], np.float32)
    edge_index = np.asarray(inputs["edge_index"])
    batch = np.asarray(inputs["batch"])
    Ws = [np.asarray(inputs[k], np.float32) for k in ("W_td1", "W_td2", "W_bu1", "W_bu2")]
    bs = [np.asarray(inputs[k], np.float32) for k in ("b_td1", "b_td2", "b_bu1", "b_bu2")]
    in_maps, meta = build_all_inputs(x, edge_index, batch, Ws, bs, cfg)
    nc = build_bass(meta)
    res = bass_utils.run_bass_kernel_spmd(
        nc, in_maps, core_ids=list(range(cfg["N_CORES"])), trace=trace)
    gpc = meta["part"]["gpc"]
    out = np.concatenate([res.results[c]["out"][:gpc] for c in range(cfg["N_CORES"])], axis=0)
    return out.astype(np.float32), res


def kernel(**inputs):
    out, _ = _run(inputs, FULL_CFG, trace=False)
    return out

